# revision 13
# baseline (speedup 1.0000x reference)
"""LocalTransformerEncoderLayer on 8 trn2 NeuronCores.

Sharding: core c = 2*b + h handles batch b, sequence half h (4096 tokens,
plus a 64-token halo on each side for the local-attention window).
Everything is done on-device per core; no collectives needed.

Layout plan (per core):
  srcT  [512, 4224] bf16  d-major haloed chunk (host-transposed)  -> QKV rhs/lhsT
  qT,kT [128,4,4224] bf16 d-major in SBUF (PE: W.T @ srcT)
  v     token-major tiles staged via DRAM scratch (PE: srcT.T @ Wv)
  per q-pair p (128 query tokens, 256 keys = ext tiles p,p+1):
    simT [128keys, 2*128q] psum  = kT.T @ qT (edge masks via rank-1; interior
    window corners zeroed post-exp by gpsimd memsets)
    expT bf16 = ACT exp(scale*simT);  denom = expT.T @ ones (PE);
    av [128q,512] = expT.T @ v;  s = av*recip + src (one DVE op); LN1 stats
  FFN per 512-token block (pipelined 3-4 pairs behind attention):
    xT via PE transpose; h[f,tok] = relu(W1.T @ xT); y[tok,d] = h.T @ W2
    residual2 + LN2 token-major, DMA out fp32.
"""
import os
import numpy as np
import ml_dtypes

_BF16 = ml_dtypes.bfloat16

B, N, D, F, W = 4, 8192, 512, 2048, 64
T = N // 2            # own tokens per core = 4096
H = 64                # halo
TEXT = T + 2 * H      # 4224
NPAIR = T // 128      # 32 q-pairs per core
NBLK = T // 512       # 8 blocks
NEG = -1e10
SCALE = float(D) ** -0.5

_cache = {}


def _build(apply_bv, apply_b2, apply_ln1g, apply_ln1b, apply_ln2g, apply_ln2b):
    import concourse.bacc as bacc
    import concourse.tile as tile
    from concourse import mybir
    import concourse.bass as bass

    f32 = mybir.dt.float32
    bf16 = mybir.dt.bfloat16
    AF = mybir.ActivationFunctionType
    ALU = mybir.AluOpType

    nc = bacc.Bacc("TRN2", target_bir_lowering=False, debug=False)

    # ---- DRAM I/O ----
    srcT_d = nc.dram_tensor("srcT", [D, TEXT], bf16, kind="ExternalInput").ap()
    src_d = nc.dram_tensor("src", [T, D], f32, kind="ExternalInput").ap()
    wq_d = nc.dram_tensor("wq", [D, D], bf16, kind="ExternalInput").ap()
    wk_d = nc.dram_tensor("wk", [D, D], bf16, kind="ExternalInput").ap()
    wv_d = nc.dram_tensor("wv", [D, D], bf16, kind="ExternalInput").ap()
    bqT_d = nc.dram_tensor("bqT", [128, 4], f32, kind="ExternalInput").ap()
    bkT_d = nc.dram_tensor("bkT", [128, 4], f32, kind="ExternalInput").ap()
    w1_d = nc.dram_tensor("w1", [D, F], bf16, kind="ExternalInput").ap()
    b1T_d = nc.dram_tensor("b1T", [128, 16], f32, kind="ExternalInput").ap()
    w2_d = nc.dram_tensor("w2", [F, D], bf16, kind="ExternalInput").ap()
    ident_d = nc.dram_tensor("ident", [128, 128], bf16, kind="ExternalInput").ap()
    uA_d = nc.dram_tensor("uA", [1, 128], bf16, kind="ExternalInput").ap()
    uB_d = nc.dram_tensor("uB", [1, 128], bf16, kind="ExternalInput").ap()
    wA0_d = nc.dram_tensor("wA0", [1, 128], bf16, kind="ExternalInput").ap()
    wB31_d = nc.dram_tensor("wB31", [1, 128], bf16, kind="ExternalInput").ap()
    if apply_bv or apply_b2:
        onerow_d = nc.dram_tensor("onerow", [1, 128], bf16, kind="ExternalInput").ap()
    if apply_bv:
        bvrow_d = nc.dram_tensor("bvrow", [1, D], bf16, kind="ExternalInput").ap()
    if apply_b2:
        b2row_d = nc.dram_tensor("b2row", [1, D], bf16, kind="ExternalInput").ap()
    # replicated LN params (only declared when needed)
    if apply_ln1g:
        g1_d = nc.dram_tensor("g1", [128, D], f32, kind="ExternalInput").ap()
    if apply_ln1b:
        be1_d = nc.dram_tensor("be1", [128, D], f32, kind="ExternalInput").ap()
    if apply_ln2g:
        g2_d = nc.dram_tensor("g2", [128, D], f32, kind="ExternalInput").ap()
    if apply_ln2b:
        be2_d = nc.dram_tensor("be2", [128, D], f32, kind="ExternalInput").ap()
    out_d = nc.dram_tensor("out", [T, D], f32, kind="ExternalOutput").ap()
    v_d = nc.dram_tensor("vscratch", [33 * 128, D], bf16).ap()

    from contextlib import ExitStack
    with tile.TileContext(nc) as tc, ExitStack() as ctx:
        # ---- persistent pools ----
        consts = ctx.enter_context(tc.tile_pool(name="consts", bufs=1))
        kv = ctx.enter_context(tc.tile_pool(name="kv", bufs=1))
        big_ps = ctx.enter_context(tc.tile_pool(name="big_ps", bufs=2, space="PSUM"))
        av_ps = ctx.enter_context(tc.tile_pool(name="av_ps", bufs=2, space="PSUM"))
        sim_ps = ctx.enter_context(tc.tile_pool(name="sim_ps", bufs=3, space="PSUM"))

        # startup-critical constants first: first matmuls need wq kt=0 +
        # srcT block-0 kt=0; split those DMAs per-kt so PE starts early
        wq_sb = consts.tile([128, 4, D], bf16, tag="wq")
        wq_r = wq_d.rearrange("(kt p) m -> p kt m", p=128)
        nc.sync.dma_start(wq_sb[:, 0], wq_r[:, 0])
        bqT_sb = consts.tile([128, 4], f32, tag="bqT")
        nc.sync.dma_start(bqT_sb, bqT_d)

        srcs = ctx.enter_context(tc.tile_pool(name="srcs", bufs=3))
        kv_io = ctx.enter_context(tc.tile_pool(name="kv_io", bufs=3))
        srcT_r = srcT_d.rearrange("(dt p) t -> p dt t", p=128)
        srcT0_sb = srcs.tile([128, 4, 512], bf16, tag="srcT")
        nc.sync.dma_start(srcT0_sb[:, 0], srcT_r[:, 0, 0:512])
        for _kt in range(1, 4):
            nc.sync.dma_start(wq_sb[:, _kt], wq_r[:, _kt])
            nc.sync.dma_start(srcT0_sb[:, _kt], srcT_r[:, _kt, 0:512])

        wk_sb = consts.tile([128, 4, D], bf16, tag="wk")
        nc.sync.dma_start(wk_sb, wk_d.rearrange("(kt p) m -> p kt m", p=128))
        bkT_sb = consts.tile([128, 4], f32, tag="bkT")
        nc.sync.dma_start(bkT_sb, bkT_d)
        wv_sb = consts.tile([128, 4, D], bf16, tag="wv")
        nc.sync.dma_start(wv_sb, wv_d.rearrange("(kt p) m -> p kt m", p=128))

        # remaining constants (small, or needed only later)
        w1_sb = consts.tile([128, 4, F], bf16, tag="w1")
        w2_sb = consts.tile([128, 16, D], bf16, tag="w2")
        b1T_sb = consts.tile([128, 16], f32, tag="b1T")
        nc.sync.dma_start(b1T_sb, b1T_d)
        ident_sb = consts.tile([128, 128], bf16, tag="ident")
        uA_sb = consts.tile([1, 128], bf16, tag="uA")
        uB_sb = consts.tile([1, 128], bf16, tag="uB")
        wA0_sb = consts.tile([1, 128], bf16, tag="wA0")
        wB31_sb = consts.tile([1, 128], bf16, tag="wB31")
        ones_sb = consts.tile([128, 1], bf16, tag="ones")
        nc.vector.memset(ones_sb, 1.0)
        eps_sb = consts.tile([128, 1], f32, tag="eps")
        nc.vector.memset(eps_sb, 1e-5)
        warm_sb = consts.tile([128, 128], bf16, tag="warm")
        nc.vector.memset(warm_sb, 0.0)
        if apply_bv:
            onerow_sb = consts.tile([1, 128], bf16, tag="onerow")
            nc.sync.dma_start(onerow_sb, onerow_d)
            bvrow_sb = consts.tile([1, D], bf16, tag="bvrow")
            nc.sync.dma_start(bvrow_sb, bvrow_d)
        if apply_b2:
            onerow2_sb = consts.tile([1, 128], bf16, tag="onerow2")
            nc.sync.dma_start(onerow2_sb, onerow_d)
            b2row_sb = consts.tile([1, D], bf16, tag="b2row")
            nc.sync.dma_start(b2row_sb, b2row_d)
        if apply_ln1g:
            g1_sb = consts.tile([128, D], f32, tag="g1")
            nc.sync.dma_start(g1_sb, g1_d)
        if apply_ln1b:
            be1_sb = consts.tile([128, D], f32, tag="be1")
            nc.sync.dma_start(be1_sb, be1_d)
        if apply_ln2g:
            g2_sb = consts.tile([128, D], f32, tag="g2")
            nc.sync.dma_start(g2_sb, g2_d)
        if apply_ln2b:
            be2_sb = consts.tile([128, D], f32, tag="be2")
            nc.sync.dma_start(be2_sb, be2_d)

        # persistent activations
        qT_sb = kv.tile([128, 4, TEXT], bf16, tag="qT")
        kT_sb = kv.tile([128, 4, TEXT], bf16, tag="kT")

        # PE warmup during the initial weight/src DMA window: ~40 dummy matmuls
        # keep the HAM activity window busy so real work starts at full clock
        for _w in range(40):
            ps_w = sim_ps.tile([128, 128], f32, tag="sim")
            nc.tensor.matmul(ps_w, lhsT=warm_sb, rhs=warm_sb, start=True, stop=True)

        # ---- phase 1: QKV over ext grid (srcT streamed per block) ----
        blocks = [(i * 512, 512) for i in range(TEXT // 512)] + [(4096, 128)]
        for off, tw in blocks:
            if off == 0:
                srcT_sb = srcT0_sb
            else:
                srcT_sb = srcs.tile([128, 4, 512], bf16, tag="srcT")
                nc.sync.dma_start(srcT_sb[:, :, :tw], srcT_r[:, :, off:off + tw])
            # qT, kT (d-major)
            for w_sb, b_sb, dst in ((wq_sb, bqT_sb, qT_sb), (wk_sb, bkT_sb, kT_sb)):
                for dq in range(4):
                    ps = big_ps.tile([128, 512], f32, tag="big")
                    for kt in range(4):
                        nc.tensor.matmul(
                            ps[:, :tw],
                            lhsT=w_sb[:, kt, dq * 128:(dq + 1) * 128],
                            rhs=srcT_sb[:, kt, :tw],
                            start=(kt == 0), stop=(kt == 3),
                        )
                    nc.scalar.activation(
                        dst[:, dq, off:off + tw], ps[:, :tw],
                        AF.Identity, bias=b_sb[:, dq:dq + 1],
                    )
            # v (token-major), per 128-token tile
            for s in range(tw // 128):
                ti = (off + s * 128) // 128
                ps = big_ps.tile([128, 512], f32, tag="big")
                for kt in range(4):
                    nc.tensor.matmul(
                        ps,
                        lhsT=srcT_sb[:, kt, s * 128:s * 128 + 128],
                        rhs=wv_sb[:, kt, :],
                        start=(kt == 0), stop=(kt == 3 and not apply_bv),
                    )
                if apply_bv:
                    nc.tensor.matmul(ps, lhsT=onerow_sb, rhs=bvrow_sb,
                                     start=False, stop=True)
                v_t = kv_io.tile([128, D], bf16, tag="vout")
                nc.vector.tensor_copy(v_t, ps)
                nc.sync.dma_start(v_d[ti * 128:(ti + 1) * 128, :], v_t)

        # small consts needed at the start of phase 2
        nc.sync.dma_start(uA_sb, uA_d)
        nc.sync.dma_start(uB_sb, uB_d)
        nc.sync.dma_start(wA0_sb, wA0_d)
        nc.sync.dma_start(wB31_sb, wB31_d)
        nc.sync.dma_start(ident_sb, ident_d)

        # ---- phase 2 pools ----
        x_pool = ctx.enter_context(tc.tile_pool(name="x_pool", bufs=9))
        # (x tiles are bf16: feed both the PE transpose and the s2 residual)
        xT_pool = ctx.enter_context(tc.tile_pool(name="xT_pool", bufs=2))
        h_pool = ctx.enter_context(tc.tile_pool(name="h_pool", bufs=1))
        io_pool = ctx.enter_context(tc.tile_pool(name="io_pool", bufs=3))
        out_pool = ctx.enter_context(tc.tile_pool(name="out_pool", bufs=2))
        stat_pool = ctx.enter_context(tc.tile_pool(name="stat_pool", bufs=8))
        vpool = ctx.enter_context(tc.tile_pool(name="vpool", bufs=4))
        exp_pool = ctx.enter_context(tc.tile_pool(name="exp_pool", bufs=4))
        s_pool = ctx.enter_context(tc.tile_pool(name="s_pool", bufs=6))
        s2_pool = ctx.enter_context(tc.tile_pool(name="s2_pool", bufs=4))

        # preload v tiles 0,1 and src tile 0 ahead of the big w1/w2 DMAs
        vts = {}
        src_ts = {}
        for t0 in (0, 1):
            vt = vpool.tile([128, D], bf16, tag="v")
            nc.sync.dma_start(vt, v_d[t0 * 128:(t0 + 1) * 128, :])
            vts[t0] = vt
        st0 = io_pool.tile([128, D], f32, tag="srct")
        nc.sync.dma_start(st0, src_d[0:128, :])
        src_ts[0] = st0

        # FFN weights needed ~10 pair-iterations into phase 2
        nc.sync.dma_start(w1_sb, w1_d.rearrange("(kt p) m -> p kt m", p=128))
        nc.sync.dma_start(w2_sb, w2_d.rearrange("(ft p) m -> p ft m", p=128))


        def ln_norm(dst, s_sb, mean_col, rstd_col, gamma, beta):
            nc.vector.tensor_scalar(dst, s_sb, mean_col, rstd_col,
                                    ALU.subtract, ALU.mult)
            if gamma is not None:
                nc.vector.tensor_mul(dst, dst, gamma)
            if beta is not None:
                nc.vector.tensor_add(dst, dst, beta)

        # ---- phase 2 pipeline state ----
        expT_t = {}
        h_blks = {}
        xbf_blks = {}
        s_tiles = {}
        mv1 = {}
        x_tiles = {}
        xT_blks = {}

        def emit_sim(p):
            qoff = H + p * 128
            if p >= 1:
                vt = vpool.tile([128, D], bf16, tag="v")
                nc.sync.dma_start(vt, v_d[(p + 1) * 128:(p + 2) * 128, :])
                vts[p + 1] = vt
                st = io_pool.tile([128, D], f32, tag="srct")
                nc.sync.dma_start(st, src_d[p * 128:(p + 1) * 128, :])
                src_ts[p] = st
            ps_sim = sim_ps.tile([128, 256], f32, tag="sim")
            for half in (0, 1):
                ktile = p + half
                edge = (uA_sb, wA0_sb) if (half == 0 and p == 0) else \
                       (uB_sb, wB31_sb) if (half == 1 and p == NPAIR - 1) else None
                reg = ps_sim[:, half * 128:(half + 1) * 128]
                for kt in range(4):
                    nc.tensor.matmul(
                        reg,
                        lhsT=kT_sb[:, kt, ktile * 128:(ktile + 1) * 128],
                        rhs=qT_sb[:, kt, qoff:qoff + 128],
                        start=(kt == 0), stop=(kt == 3 and edge is None),
                    )
                if edge is not None:
                    nc.tensor.matmul(reg, lhsT=edge[0], rhs=edge[1],
                                     start=False, stop=True)
            expT = exp_pool.tile([128, 256], bf16, tag="expT")
            nc.scalar.activation(expT, ps_sim, AF.Exp, scale=SCALE)
            # interior window-corner masks: zero after exp (cheaper than rank-1 matmuls)
            if p > 0:
                nc.gpsimd.memset(expT[0:64, 64:128], 0.0)
            if p < NPAIR - 1:
                nc.gpsimd.memset(expT[64:128, 128:192], 0.0)
            expT_t[p] = expT

        def emit_av(p):
            expT = expT_t.pop(p)
            vA = vts.pop(p)
            vB = vts[p + 1]
            ps_den = sim_ps.tile([128, 1], f32, tag="sim")
            nc.tensor.matmul(ps_den, lhsT=expT[:, 0:128], rhs=ones_sb,
                             start=True, stop=False)
            nc.tensor.matmul(ps_den, lhsT=expT[:, 128:256], rhs=ones_sb,
                             start=False, stop=True)
            recip = stat_pool.tile([128, 1], f32, tag="recip")
            nc.vector.reciprocal(recip, ps_den)
            ps_av = av_ps.tile([128, 512], f32, tag="av")
            nc.tensor.matmul(ps_av, lhsT=expT[:, 0:128], rhs=vA,
                             start=True, stop=False)
            nc.tensor.matmul(ps_av, lhsT=expT[:, 128:256], rhs=vB,
                             start=False, stop=True)
            # s = av*recip + src in a single DVE op
            s_sb = s_pool.tile([128, D], f32, tag="s")
            nc.vector.scalar_tensor_tensor(s_sb, ps_av, recip, src_ts.pop(p),
                                           ALU.mult, ALU.add)
            # per-pair LN1: stats -> rstd -> normalized bf16 x (feeds FFN + residual2)
            st6 = stat_pool.tile([128, 6], f32, tag="st6")
            nc.vector.bn_stats(st6, s_sb)
            mv = stat_pool.tile([128, 2], f32, tag="mv1")
            nc.vector.bn_aggr(mv, st6)
            std = stat_pool.tile([128, 1], f32, tag="std1")
            nc.scalar.activation(std, mv[:, 1:2], AF.Sqrt, bias=eps_sb)
            rstd = stat_pool.tile([128, 1], f32, tag="rstd1")
            nc.vector.reciprocal(rstd, std)
            x_bf = x_pool.tile([128, D], bf16, tag="xbf")
            ln_norm(x_bf, s_sb, mv[:, 0:1], rstd,
                    g1_sb if apply_ln1g else None,
                    be1_sb if apply_ln1b else None)
            x_tiles[p] = x_bf


        def emit_transposes(blk):
            xT_blk = xT_pool.tile([128, 4, 512], bf16, tag="xT")
            xT_blks[blk] = xT_blk
            for j in range(4):
                xbf = x_tiles[blk * 4 + j]
                for dt in range(4):
                    ps_xt = sim_ps.tile([128, 128], bf16, tag="sim")
                    nc.tensor.transpose(ps_xt, xbf[:, dt * 128:(dt + 1) * 128],
                                        ident_sb)
                    nc.vector.tensor_copy(
                        xT_blk[:, dt, j * 128:(j + 1) * 128], ps_xt)

        def emit_ffn_h(blk):
            xT_blk = xT_blks.pop(blk)
            h_sb = h_pool.tile([128, 16, 512], bf16, tag="h")
            for ft in range(16):
                ps_h = big_ps.tile([128, 512], f32, tag="big")
                for kt in range(4):
                    nc.tensor.matmul(
                        ps_h,
                        lhsT=w1_sb[:, kt, ft * 128:(ft + 1) * 128],
                        rhs=xT_blk[:, kt, :],
                        start=(kt == 0), stop=(kt == 3),
                    )
                nc.vector.tensor_scalar(h_sb[:, ft, :], ps_h,
                                        b1T_sb[:, ft:ft + 1], 0.0,
                                        ALU.add, ALU.max)
            h_blks[blk] = h_sb

        def emit_ffn_y(blk):
            h_sb = h_blks.pop(blk)
            for j in range(4):
                p = blk * 4 + j
                ps_y = big_ps.tile([128, 512], f32, tag="big")
                for ft in range(16):
                    nc.tensor.matmul(
                        ps_y,
                        lhsT=h_sb[:, ft, j * 128:(j + 1) * 128],
                        rhs=w2_sb[:, ft, :],
                        start=(ft == 0), stop=(ft == 15 and not apply_b2),
                    )
                if apply_b2:
                    nc.tensor.matmul(ps_y, lhsT=onerow2_sb, rhs=b2row_sb,
                                     start=False, stop=True)
                s2 = s2_pool.tile([128, D], f32, tag="s2")
                nc.vector.tensor_add(s2, x_tiles.pop(p), ps_y)
                st6 = stat_pool.tile([128, 6], f32, tag="st6")
                nc.vector.bn_stats(st6, s2)
                mv2 = stat_pool.tile([128, 2], f32, tag="mv2")
                nc.vector.bn_aggr(mv2, st6)
                std2 = stat_pool.tile([128, 1], f32, tag="std2")
                nc.scalar.activation(std2, mv2[:, 1:2], AF.Sqrt, bias=eps_sb)
                rstd2 = stat_pool.tile([128, 1], f32, tag="rstd2")
                nc.vector.reciprocal(rstd2, std2)
                o_sb = out_pool.tile([128, D], f32, tag="o")
                ln_norm(o_sb, s2, mv2[:, 0:1], rstd2,
                        g2_sb if apply_ln2g else None,
                        be2_sb if apply_ln2b else None)
                nc.sync.dma_start(out_d[p * 128:(p + 1) * 128, :], o_sb)

        # ---- phase 2 pipeline ----
        # av(p-1) | sim(p) | T(blk) at 4b+7 | LN1(blk) at 4b+5 | H at 4b+8 | Y at 4b+9
        for p in range(NPAIR + 6):
            if p < NPAIR:
                emit_sim(p)
            if 1 <= p <= NPAIR:
                emit_av(p - 1)
            if p >= 7 and (p - 7) % 4 == 0 and (p - 7) // 4 < NBLK:
                emit_transposes((p - 7) // 4)
            if p >= 8 and (p - 8) % 4 == 0:
                emit_ffn_h((p - 8) // 4)
            if p >= 9 and (p - 9) % 4 == 0:
                emit_ffn_y((p - 9) // 4)

    nc.compile()
    return nc


def _get_program(key):
    if key not in _cache:
        _cache[key] = _build(*key)
    return _cache[key]


last_exec_ns = None


def _install_ntff_hook():
    """NTFF profiling hook for axon (normally installed via antenv.axon_hooks)."""
    import sys, types
    if 'antenv.axon_hooks' in sys.modules:
        return
    mod = types.ModuleType('antenv.axon_hooks')
    _h = [None]
    mod.set_axon_ntff_profile_hook = lambda h: _h.__setitem__(0, h)
    mod.get_axon_ntff_profile_hook = lambda: _h[0]
    sys.modules['antenv.axon_hooks'] = mod
    import antenv
    antenv.axon_hooks = mod
    try:
        from trn_agent_boot.trn_boot import _ntff_profile_via_ctypes
        mod.set_axon_ntff_profile_hook(
            _ntff_profile_via_ctypes('/opt/axon/libaxon_pjrt.so'))
    except Exception:
        pass


def kernel(src, mask, Wq, bq, Wk, bk, Wv, bv, ln1_g, ln1_b,
           W1, b1, W2, b2, ln2_g, ln2_b):
    global last_exec_ns
    src = np.asarray(src, np.float32)
    if not bool(np.asarray(mask).all()):
        raise NotImplementedError("only all-true mask supported")

    key = (bool(np.any(bv)), bool(np.any(b2)),
           not bool(np.all(ln1_g == 1)), bool(np.any(ln1_b)),
           not bool(np.all(ln2_g == 1)), bool(np.any(ln2_b)))
    nc = _get_program(key)
    apply_bv, apply_b2, a_g1, a_b1, a_g2, a_b2 = key

    qi = np.arange(128)
    wA = np.where(qi >= 64, NEG, 0.0).astype(_BF16).reshape(1, 128)
    wB = np.where(qi < 64, NEG, 0.0).astype(_BF16).reshape(1, 128)
    wfull = np.full((1, 128), NEG, _BF16)
    uA = (qi < 64).astype(_BF16).reshape(1, 128)
    uB = (qi >= 64).astype(_BF16).reshape(1, 128)

    shared = {
        "wq": Wq.astype(_BF16), "wk": Wk.astype(_BF16), "wv": Wv.astype(_BF16),
        "bqT": np.asarray(bq, np.float32).reshape(4, 128).T.copy(),
        "bkT": np.asarray(bk, np.float32).reshape(4, 128).T.copy(),
        "w1": W1.astype(_BF16),
        "b1T": np.asarray(b1, np.float32).reshape(16, 128).T.copy(),
        "w2": W2.astype(_BF16),
        "ident": np.eye(128, dtype=_BF16),
        "uA": uA, "uB": uB,
    }
    if apply_bv or apply_b2:
        shared["onerow"] = np.ones((1, 128), _BF16)
    if apply_bv:
        shared["bvrow"] = np.asarray(bv, np.float32).reshape(1, D).astype(_BF16)
    if apply_b2:
        shared["b2row"] = np.asarray(b2, np.float32).reshape(1, D).astype(_BF16)
    if a_g1:
        shared["g1"] = np.tile(np.asarray(ln1_g, np.float32).reshape(1, D), (128, 1))
    if a_b1:
        shared["be1"] = np.tile(np.asarray(ln1_b, np.float32).reshape(1, D), (128, 1))
    if a_g2:
        shared["g2"] = np.tile(np.asarray(ln2_g, np.float32).reshape(1, D), (128, 1))
    if a_b2:
        shared["be2"] = np.tile(np.asarray(ln2_b, np.float32).reshape(1, D), (128, 1))

    in_maps = []
    for c in range(8):
        b, h = divmod(c, 2)
        start = h * T - H
        ext = np.zeros((TEXT, D), np.float32)
        lo, hi = max(start, 0), min(start + TEXT, N)
        ext[lo - start: hi - start] = src[b, lo:hi]
        m = dict(shared)
        m["srcT"] = np.ascontiguousarray(ext.T).astype(_BF16)
        m["src"] = np.ascontiguousarray(src[b, h * T:(h + 1) * T])
        m["wA0"] = wfull if h == 0 else wA
        m["wB31"] = wfull if h == 1 else wB
        in_maps.append(m)

    from concourse.bass_utils import run_bass_kernel_spmd
    trace = bool(os.environ.get("KERNEL_TRACE"))
    if trace:
        _install_ntff_hook()
    res = run_bass_kernel_spmd(nc, in_maps, core_ids=list(range(8)), trace=trace)
    if trace:
        last_exec_ns = res.exec_time_ns

    out = np.empty((B, N, D), np.float32)
    for c in range(8):
        b, h = divmod(c, 2)
        out[b, h * T:(h + 1) * T] = res.results[c]["out"]
    return out


# revision 17
# speedup vs baseline: 1.0300x; 1.0300x over previous
"""LocalTransformerEncoderLayer on 8 trn2 NeuronCores.

Sharding: core c = 2*b + h handles batch b, sequence half h (4096 tokens,
plus a 64-token halo on each side for the local-attention window).
Everything is done on-device per core; no collectives needed.

Layout plan (per core):
  srcT  [512, 4224] bf16  d-major haloed chunk (host-transposed)  -> QKV rhs/lhsT
  qT,kT [128,4,4224] bf16 d-major in SBUF (PE: W.T @ srcT)
  v     token-major tiles staged via DRAM scratch (PE: srcT.T @ Wv)
  per q-pair p (128 query tokens, 256 keys = ext tiles p,p+1):
    simT [128keys, 2*128q] psum  = kT.T @ qT (edge masks via rank-1; interior
    window corners zeroed post-exp by gpsimd memsets)
    expT bf16 = ACT exp(scale*simT);  denom = expT.T @ ones (PE);
    av [128q,512] = expT.T @ v;  s = av*recip + src (one DVE op); LN1 stats
  FFN per 512-token block (pipelined 3-4 pairs behind attention):
    xT via PE transpose; h[f,tok] = relu(W1.T @ xT); y[tok,d] = h.T @ W2
    residual2 + LN2 token-major, DMA out fp32.
"""
import os
import numpy as np
import ml_dtypes

_BF16 = ml_dtypes.bfloat16

B, N, D, F, W = 4, 8192, 512, 2048, 64
T = N // 2            # own tokens per core = 4096
H = 64                # halo
TEXT = T + 2 * H      # 4224
NPAIR = T // 128      # 32 q-pairs per core
NBLK = T // 512       # 8 blocks
NEG = -1e10
SCALE = float(D) ** -0.5

_cache = {}


def _build(apply_bv, apply_b2, apply_ln1g, apply_ln1b, apply_ln2g, apply_ln2b):
    import concourse.bacc as bacc
    import concourse.tile as tile
    from concourse import mybir
    import concourse.bass as bass

    f32 = mybir.dt.float32
    bf16 = mybir.dt.bfloat16
    AF = mybir.ActivationFunctionType
    ALU = mybir.AluOpType

    nc = bacc.Bacc("TRN2", target_bir_lowering=False, debug=False)

    # ---- DRAM I/O ----
    srcT_d = nc.dram_tensor("srcT", [D, TEXT], bf16, kind="ExternalInput").ap()
    src_d = nc.dram_tensor("src", [T, D], f32, kind="ExternalInput").ap()
    wq_d = nc.dram_tensor("wq", [D, D], bf16, kind="ExternalInput").ap()
    wk_d = nc.dram_tensor("wk", [D, D], bf16, kind="ExternalInput").ap()
    wv_d = nc.dram_tensor("wv", [D, D], bf16, kind="ExternalInput").ap()
    bqT_d = nc.dram_tensor("bqT", [128, 4], f32, kind="ExternalInput").ap()
    bkT_d = nc.dram_tensor("bkT", [128, 4], f32, kind="ExternalInput").ap()
    w1_d = nc.dram_tensor("w1", [D, F], bf16, kind="ExternalInput").ap()
    b1T_d = nc.dram_tensor("b1T", [128, 16], f32, kind="ExternalInput").ap()
    w2_d = nc.dram_tensor("w2", [F, D], bf16, kind="ExternalInput").ap()
    ident_d = nc.dram_tensor("ident", [128, 128], bf16, kind="ExternalInput").ap()
    uA_d = nc.dram_tensor("uA", [1, 128], bf16, kind="ExternalInput").ap()
    uB_d = nc.dram_tensor("uB", [1, 128], bf16, kind="ExternalInput").ap()
    wA0_d = nc.dram_tensor("wA0", [1, 128], bf16, kind="ExternalInput").ap()
    wB31_d = nc.dram_tensor("wB31", [1, 128], bf16, kind="ExternalInput").ap()
    if apply_bv or apply_b2:
        onerow_d = nc.dram_tensor("onerow", [1, 128], bf16, kind="ExternalInput").ap()
    if apply_bv:
        bvrow_d = nc.dram_tensor("bvrow", [1, D], bf16, kind="ExternalInput").ap()
    if apply_b2:
        b2row_d = nc.dram_tensor("b2row", [1, D], bf16, kind="ExternalInput").ap()
    # replicated LN params (only declared when needed)
    if apply_ln1g:
        g1_d = nc.dram_tensor("g1", [128, D], f32, kind="ExternalInput").ap()
    if apply_ln1b:
        be1_d = nc.dram_tensor("be1", [128, D], f32, kind="ExternalInput").ap()
    if apply_ln2g:
        g2_d = nc.dram_tensor("g2", [128, D], f32, kind="ExternalInput").ap()
    if apply_ln2b:
        be2_d = nc.dram_tensor("be2", [128, D], f32, kind="ExternalInput").ap()
    out_d = nc.dram_tensor("out", [T, D], f32, kind="ExternalOutput").ap()
    v_d = nc.dram_tensor("vscratch", [33 * 128, D], bf16).ap()

    from contextlib import ExitStack
    with tile.TileContext(nc) as tc, ExitStack() as ctx:
        # ---- persistent pools ----
        consts = ctx.enter_context(tc.tile_pool(name="consts", bufs=1))
        kv = ctx.enter_context(tc.tile_pool(name="kv", bufs=1))
        big_ps = ctx.enter_context(tc.tile_pool(name="big_ps", bufs=2, space="PSUM"))
        av_ps = ctx.enter_context(tc.tile_pool(name="av_ps", bufs=2, space="PSUM"))
        sim_ps = ctx.enter_context(tc.tile_pool(name="sim_ps", bufs=3, space="PSUM"))

        # startup-critical constants first: first matmuls need wq kt=0 +
        # srcT block-0 kt=0; split those DMAs per-kt so PE starts early
        wq_sb = consts.tile([128, 4, D], bf16, tag="wq")
        wq_r = wq_d.rearrange("(kt p) m -> p kt m", p=128)
        nc.sync.dma_start(wq_sb[:, 0], wq_r[:, 0])
        bqT_sb = consts.tile([128, 4], f32, tag="bqT")
        nc.sync.dma_start(bqT_sb, bqT_d)

        srcs = ctx.enter_context(tc.tile_pool(name="srcs", bufs=3))
        kv_io = ctx.enter_context(tc.tile_pool(name="kv_io", bufs=3))
        srcT_r = srcT_d.rearrange("(dt p) t -> p dt t", p=128)
        srcT0_sb = srcs.tile([128, 4, 512], bf16, tag="srcT")
        nc.sync.dma_start(srcT0_sb[:, 0], srcT_r[:, 0, 0:512])
        for _kt in range(1, 4):
            nc.sync.dma_start(wq_sb[:, _kt], wq_r[:, _kt])
            nc.sync.dma_start(srcT0_sb[:, _kt], srcT_r[:, _kt, 0:512])

        wk_sb = consts.tile([128, 4, D], bf16, tag="wk")
        nc.sync.dma_start(wk_sb, wk_d.rearrange("(kt p) m -> p kt m", p=128))
        bkT_sb = consts.tile([128, 4], f32, tag="bkT")
        nc.sync.dma_start(bkT_sb, bkT_d)
        wv_sb = consts.tile([128, 4, D], bf16, tag="wv")
        nc.sync.dma_start(wv_sb, wv_d.rearrange("(kt p) m -> p kt m", p=128))

        # remaining constants (small, or needed only later)
        w1_sb = consts.tile([128, 4, F], bf16, tag="w1")
        w2_sb = consts.tile([128, 16, D], bf16, tag="w2")
        b1T_sb = consts.tile([128, 16], f32, tag="b1T")
        nc.sync.dma_start(b1T_sb, b1T_d)
        ident_sb = consts.tile([128, 128], bf16, tag="ident")
        uA_sb = consts.tile([1, 128], bf16, tag="uA")
        uB_sb = consts.tile([1, 128], bf16, tag="uB")
        wA0_sb = consts.tile([1, 128], bf16, tag="wA0")
        wB31_sb = consts.tile([1, 128], bf16, tag="wB31")
        ones_sb = consts.tile([128, 1], bf16, tag="ones")
        nc.vector.memset(ones_sb, 1.0)
        eps_sb = consts.tile([128, 1], f32, tag="eps")
        nc.vector.memset(eps_sb, 1e-5)
        warm_sb = consts.tile([128, 128], bf16, tag="warm")
        nc.vector.memset(warm_sb, 0.0)
        if apply_bv:
            onerow_sb = consts.tile([1, 128], bf16, tag="onerow")
            nc.sync.dma_start(onerow_sb, onerow_d)
            bvrow_sb = consts.tile([1, D], bf16, tag="bvrow")
            nc.sync.dma_start(bvrow_sb, bvrow_d)
        if apply_b2:
            onerow2_sb = consts.tile([1, 128], bf16, tag="onerow2")
            nc.sync.dma_start(onerow2_sb, onerow_d)
            b2row_sb = consts.tile([1, D], bf16, tag="b2row")
            nc.sync.dma_start(b2row_sb, b2row_d)
        if apply_ln1g:
            g1_sb = consts.tile([128, D], f32, tag="g1")
            nc.sync.dma_start(g1_sb, g1_d)
        if apply_ln1b:
            be1_sb = consts.tile([128, D], f32, tag="be1")
            nc.sync.dma_start(be1_sb, be1_d)
        if apply_ln2g:
            g2_sb = consts.tile([128, D], f32, tag="g2")
            nc.sync.dma_start(g2_sb, g2_d)
        if apply_ln2b:
            be2_sb = consts.tile([128, D], f32, tag="be2")
            nc.sync.dma_start(be2_sb, be2_d)

        # persistent activations
        qT_sb = kv.tile([128, 4, TEXT], bf16, tag="qT")
        kT_sb = kv.tile([128, 4, TEXT], bf16, tag="kT")

        # PE warmup during the initial weight/src DMA window: ~40 dummy matmuls
        # keep the HAM activity window busy so real work starts at full clock
        for _w in range(40):
            ps_w = sim_ps.tile([128, 128], f32, tag="sim")
            nc.tensor.matmul(ps_w, lhsT=warm_sb, rhs=warm_sb, start=True, stop=True)

        # ---- phase 1: QKV over ext grid (srcT streamed per block) ----
        blocks = [(i * 512, 512) for i in range(TEXT // 512)] + [(4096, 128)]
        for off, tw in blocks:
            if off == 0:
                srcT_sb = srcT0_sb
            else:
                srcT_sb = srcs.tile([128, 4, 512], bf16, tag="srcT")
                nc.sync.dma_start(srcT_sb[:, :, :tw], srcT_r[:, :, off:off + tw])
            # qT, kT (d-major)
            for w_sb, b_sb, dst in ((wq_sb, bqT_sb, qT_sb), (wk_sb, bkT_sb, kT_sb)):
                for dq in range(4):
                    ps = big_ps.tile([128, 512], f32, tag="big")
                    for kt in range(4):
                        nc.tensor.matmul(
                            ps[:, :tw],
                            lhsT=w_sb[:, kt, dq * 128:(dq + 1) * 128],
                            rhs=srcT_sb[:, kt, :tw],
                            start=(kt == 0), stop=(kt == 3),
                        )
                    nc.scalar.activation(
                        dst[:, dq, off:off + tw], ps[:, :tw],
                        AF.Identity, bias=b_sb[:, dq:dq + 1],
                    )
            # v (token-major), per 128-token tile
            for s in range(tw // 128):
                ti = (off + s * 128) // 128
                ps = big_ps.tile([128, 512], f32, tag="big")
                for kt in range(4):
                    nc.tensor.matmul(
                        ps,
                        lhsT=srcT_sb[:, kt, s * 128:s * 128 + 128],
                        rhs=wv_sb[:, kt, :],
                        start=(kt == 0), stop=(kt == 3 and not apply_bv),
                    )
                if apply_bv:
                    nc.tensor.matmul(ps, lhsT=onerow_sb, rhs=bvrow_sb,
                                     start=False, stop=True)
                v_t = kv_io.tile([128, D], bf16, tag="vout")
                nc.vector.tensor_copy(v_t, ps)
                nc.sync.dma_start(v_d[ti * 128:(ti + 1) * 128, :], v_t)

        # small consts needed at the start of phase 2
        nc.sync.dma_start(uA_sb, uA_d)
        nc.sync.dma_start(uB_sb, uB_d)
        nc.sync.dma_start(wA0_sb, wA0_d)
        nc.sync.dma_start(wB31_sb, wB31_d)
        nc.sync.dma_start(ident_sb, ident_d)

        # ---- phase 2 pools ----
        x_pool = ctx.enter_context(tc.tile_pool(name="x_pool", bufs=9))
        # (x tiles are bf16: feed both the PE transpose and the s2 residual)
        xT_pool = ctx.enter_context(tc.tile_pool(name="xT_pool", bufs=2))
        h_pool = ctx.enter_context(tc.tile_pool(name="h_pool", bufs=1))
        io_pool = ctx.enter_context(tc.tile_pool(name="io_pool", bufs=3))
        out_pool = ctx.enter_context(tc.tile_pool(name="out_pool", bufs=2))
        stat_pool = ctx.enter_context(tc.tile_pool(name="stat_pool", bufs=8))
        vpool = ctx.enter_context(tc.tile_pool(name="vpool", bufs=4))
        exp_pool = ctx.enter_context(tc.tile_pool(name="exp_pool", bufs=4))
        s_pool = ctx.enter_context(tc.tile_pool(name="s_pool", bufs=6))
        s2_pool = ctx.enter_context(tc.tile_pool(name="s2_pool", bufs=4))

        # preload v tiles 0,1 and src tile 0 ahead of the big w1/w2 DMAs
        vts = {}
        src_ts = {}
        for t0 in (0, 1):
            vt = vpool.tile([128, D], bf16, tag="v")
            nc.sync.dma_start(vt, v_d[t0 * 128:(t0 + 1) * 128, :])
            vts[t0] = vt
        st0 = io_pool.tile([128, D], f32, tag="srct")
        nc.sync.dma_start(st0, src_d[0:128, :])
        src_ts[0] = st0

        # FFN weights needed ~10 pair-iterations into phase 2
        nc.sync.dma_start(w1_sb, w1_d.rearrange("(kt p) m -> p kt m", p=128))
        nc.sync.dma_start(w2_sb, w2_d.rearrange("(ft p) m -> p ft m", p=128))


        u32 = mybir.dt.uint32
        MAGIC1 = 0x5f375a86 + 1

        def rsqrt_dve(mv_blk, n, tag):
            """rstd[128,n] = rsqrt(var+eps) on DVE: bit-hack seed + 2 Newton.

            mv_blk packs (mean, var) pairs; vars live at odd columns. Keeps
            Sqrt off the Scalar engine so its ACT table never thrashes
            between Exp/Relu and Sqrt (each reload costs 1.28us).
            """
            if n == 1:
                var_view = mv_blk[:, 1:2]
            else:
                var_view = mv_blk.rearrange("p (n two) -> p n two", two=2)[:, :, 1]
            veps = stat_pool.tile([128, n], f32, tag=tag + "ve", name="veps")
            nc.vector.tensor_scalar_add(veps, var_view, 1e-5)
            # seed bits = MAGIC - (bits(x) >> 1); the subtract must run in the
            # f32 value domain (DVE int add saturates instead of wrapping)
            a = stat_pool.tile([128, n], u32, tag=tag + "a", name="rsq_a")
            nc.vector.tensor_scalar(a, veps.bitcast(u32), 1, None,
                                    ALU.logical_shift_right)
            af = stat_pool.tile([128, n], f32, tag=tag + "af", name="rsq_af")
            nc.vector.tensor_copy(af, a)
            nc.vector.tensor_scalar(af, af, -1.0, float(MAGIC1 - 1),
                                    ALU.mult, ALU.add)
            yb = stat_pool.tile([128, n], u32, tag=tag + "y", name="rsq_y")
            nc.vector.tensor_copy(yb, af)
            y = yb.bitcast(f32)
            t = stat_pool.tile([128, n], f32, tag=tag + "t", name="rsq_t")
            for _ in range(2):
                nc.vector.tensor_tensor(t, y, y, ALU.mult)
                nc.vector.scalar_tensor_tensor(t, t, -0.5, veps, ALU.mult, ALU.mult)
                nc.vector.scalar_tensor_tensor(y, t, 1.5, y, ALU.add, ALU.mult)
            return y

        def ln_norm(dst, s_sb, mean_col, rstd_col, gamma, beta):
            nc.vector.tensor_scalar(dst, s_sb, mean_col, rstd_col,
                                    ALU.subtract, ALU.mult)
            if gamma is not None:
                nc.vector.tensor_mul(dst, dst, gamma)
            if beta is not None:
                nc.vector.tensor_add(dst, dst, beta)

        # ---- phase 2 pipeline state ----
        expT_t = {}
        h_blks = {}
        xbf_blks = {}
        s_tiles = {}
        mv1 = {}
        x_tiles = {}
        xT_blks = {}

        def emit_sim(p):
            qoff = H + p * 128
            if p >= 1:
                vt = vpool.tile([128, D], bf16, tag="v")
                nc.sync.dma_start(vt, v_d[(p + 1) * 128:(p + 2) * 128, :])
                vts[p + 1] = vt
                st = io_pool.tile([128, D], f32, tag="srct")
                nc.sync.dma_start(st, src_d[p * 128:(p + 1) * 128, :])
                src_ts[p] = st
            ps_sim = sim_ps.tile([128, 256], f32, tag="sim")
            for half in (0, 1):
                ktile = p + half
                edge = (uA_sb, wA0_sb) if (half == 0 and p == 0) else \
                       (uB_sb, wB31_sb) if (half == 1 and p == NPAIR - 1) else None
                reg = ps_sim[:, half * 128:(half + 1) * 128]
                for kt in range(4):
                    nc.tensor.matmul(
                        reg,
                        lhsT=kT_sb[:, kt, ktile * 128:(ktile + 1) * 128],
                        rhs=qT_sb[:, kt, qoff:qoff + 128],
                        start=(kt == 0), stop=(kt == 3 and edge is None),
                    )
                if edge is not None:
                    nc.tensor.matmul(reg, lhsT=edge[0], rhs=edge[1],
                                     start=False, stop=True)
            expT = exp_pool.tile([128, 256], bf16, tag="expT")
            nc.scalar.activation(expT, ps_sim, AF.Exp, scale=SCALE)
            # interior window-corner masks: zero after exp (cheaper than rank-1 matmuls)
            if p > 0:
                nc.gpsimd.memset(expT[0:64, 64:128], 0.0)
            if p < NPAIR - 1:
                nc.gpsimd.memset(expT[64:128, 128:192], 0.0)
            expT_t[p] = expT

        def emit_av(p):
            expT = expT_t.pop(p)
            vA = vts.pop(p)
            vB = vts[p + 1]
            ps_den = sim_ps.tile([128, 1], f32, tag="sim")
            nc.tensor.matmul(ps_den, lhsT=expT[:, 0:128], rhs=ones_sb,
                             start=True, stop=False)
            nc.tensor.matmul(ps_den, lhsT=expT[:, 128:256], rhs=ones_sb,
                             start=False, stop=True)
            recip = stat_pool.tile([128, 1], f32, tag="recip")
            nc.vector.reciprocal(recip, ps_den)
            ps_av = av_ps.tile([128, 512], f32, tag="av")
            nc.tensor.matmul(ps_av, lhsT=expT[:, 0:128], rhs=vA,
                             start=True, stop=False)
            nc.tensor.matmul(ps_av, lhsT=expT[:, 128:256], rhs=vB,
                             start=False, stop=True)
            # s = av*recip + src in a single DVE op
            s_sb = s_pool.tile([128, D], f32, tag="s")
            nc.vector.scalar_tensor_tensor(s_sb, ps_av, recip, src_ts.pop(p),
                                           ALU.mult, ALU.add)
            s_tiles[p] = s_sb
            blk, j = divmod(p, 4)
            if j == 0:
                mv1[blk] = stat_pool.tile([128, 8], f32, tag="mv1b", name="mv1b")
            st6 = stat_pool.tile([128, 6], f32, tag="st6")
            nc.vector.bn_stats(st6, s_sb)
            nc.vector.bn_aggr(mv1[blk][:, 2 * j:2 * j + 2], st6)


        def ln_finish(blk):
            mv_blk = mv1.pop(blk)
            rstd1 = rsqrt_dve(mv_blk, 4, "r1")
            for j in range(4):
                p = blk * 4 + j
                x_bf = x_pool.tile([128, D], bf16, tag="xbf")
                ln_norm(x_bf, s_tiles.pop(p), mv_blk[:, 2 * j:2 * j + 1],
                        rstd1[:, j:j + 1],
                        g1_sb if apply_ln1g else None,
                        be1_sb if apply_ln1b else None)
                x_tiles[p] = x_bf

        def emit_transposes(blk):
            xT_blk = xT_pool.tile([128, 4, 512], bf16, tag="xT")
            xT_blks[blk] = xT_blk
            for j in range(4):
                xbf = x_tiles[blk * 4 + j]
                ps_xt = sim_ps.tile([128, 4, 128], bf16, tag="sim")
                for dt in range(4):
                    nc.tensor.transpose(ps_xt[:, dt], xbf[:, dt * 128:(dt + 1) * 128],
                                        ident_sb)
                nc.vector.tensor_copy(xT_blk[:, :, j * 128:(j + 1) * 128], ps_xt)

        def emit_ffn_h(blk):
            xT_blk = xT_blks.pop(blk)
            h_sb = h_pool.tile([128, 16, 512], bf16, tag="h")
            for ft in range(16):
                ps_h = big_ps.tile([128, 512], f32, tag="big")
                for kt in range(4):
                    nc.tensor.matmul(
                        ps_h,
                        lhsT=w1_sb[:, kt, ft * 128:(ft + 1) * 128],
                        rhs=xT_blk[:, kt, :],
                        start=(kt == 0), stop=(kt == 3),
                    )
                nc.scalar.activation(h_sb[:, ft, :], ps_h, AF.Relu,
                                     bias=b1T_sb[:, ft:ft + 1])
            h_blks[blk] = h_sb

        def emit_ffn_y(blk):
            h_sb = h_blks.pop(blk)
            stream = blk == NBLK - 1
            s2_tiles = []
            mv2_blk = stat_pool.tile([128, 8], f32, tag="mv2b", name="mv2b")
            for j in range(4):
                p = blk * 4 + j
                ps_y = big_ps.tile([128, 512], f32, tag="big")
                for ft in range(16):
                    nc.tensor.matmul(
                        ps_y,
                        lhsT=h_sb[:, ft, j * 128:(j + 1) * 128],
                        rhs=w2_sb[:, ft, :],
                        start=(ft == 0), stop=(ft == 15 and not apply_b2),
                    )
                if apply_b2:
                    nc.tensor.matmul(ps_y, lhsT=onerow2_sb, rhs=b2row_sb,
                                     start=False, stop=True)
                s2 = s2_pool.tile([128, D], f32, tag="s2")
                nc.vector.tensor_add(s2, x_tiles.pop(p), ps_y)
                s2_tiles.append(s2)
                st6 = stat_pool.tile([128, 6], f32, tag="st6")
                nc.vector.bn_stats(st6, s2)
                nc.vector.bn_aggr(mv2_blk[:, 2 * j:2 * j + 2], st6)
                if stream:
                    rstd_j = rsqrt_dve(mv2_blk[:, 2 * j:2 * j + 2], 1, "r2")
                    o_sb = out_pool.tile([128, D], f32, tag="o")
                    ln_norm(o_sb, s2, mv2_blk[:, 2 * j:2 * j + 1], rstd_j,
                            g2_sb if apply_ln2g else None,
                            be2_sb if apply_ln2b else None)
                    nc.sync.dma_start(out_d[p * 128:(p + 1) * 128, :], o_sb)
            if stream:
                return
            rstd2 = rsqrt_dve(mv2_blk, 4, "r2")
            for j in range(4):
                p = blk * 4 + j
                o_sb = out_pool.tile([128, D], f32, tag="o")
                ln_norm(o_sb, s2_tiles[j], mv2_blk[:, 2 * j:2 * j + 1],
                        rstd2[:, j:j + 1],
                        g2_sb if apply_ln2g else None,
                        be2_sb if apply_ln2b else None)
                nc.sync.dma_start(out_d[p * 128:(p + 1) * 128, :], o_sb)

        # ---- phase 2 pipeline ----
        # av(p-1) | sim(p) | T(blk) at 4b+7 | LN1(blk) at 4b+5 | H at 4b+8 | Y at 4b+9
        for p in range(NPAIR + 6):
            if p < NPAIR:
                emit_sim(p)
            if 1 <= p <= NPAIR:
                emit_av(p - 1)
            if p >= 5 and (p - 5) % 4 == 0 and (p - 5) // 4 < NBLK:
                ln_finish((p - 5) // 4)
            if p >= 7 and (p - 7) % 4 == 0 and (p - 7) // 4 < NBLK:
                emit_transposes((p - 7) // 4)
            if p >= 8 and (p - 8) % 4 == 0:
                emit_ffn_h((p - 8) // 4)
            if p >= 9 and (p - 9) % 4 == 0:
                emit_ffn_y((p - 9) // 4)

    nc.compile()
    return nc


def _get_program(key):
    if key not in _cache:
        _cache[key] = _build(*key)
    return _cache[key]


last_exec_ns = None


def _install_ntff_hook():
    """NTFF profiling hook for axon (normally installed via antenv.axon_hooks)."""
    import sys, types
    if 'antenv.axon_hooks' in sys.modules:
        return
    mod = types.ModuleType('antenv.axon_hooks')
    _h = [None]
    mod.set_axon_ntff_profile_hook = lambda h: _h.__setitem__(0, h)
    mod.get_axon_ntff_profile_hook = lambda: _h[0]
    sys.modules['antenv.axon_hooks'] = mod
    import antenv
    antenv.axon_hooks = mod
    try:
        from trn_agent_boot.trn_boot import _ntff_profile_via_ctypes
        mod.set_axon_ntff_profile_hook(
            _ntff_profile_via_ctypes('/opt/axon/libaxon_pjrt.so'))
    except Exception:
        pass


def kernel(src, mask, Wq, bq, Wk, bk, Wv, bv, ln1_g, ln1_b,
           W1, b1, W2, b2, ln2_g, ln2_b):
    global last_exec_ns
    src = np.asarray(src, np.float32)
    if not bool(np.asarray(mask).all()):
        raise NotImplementedError("only all-true mask supported")

    key = (bool(np.any(bv)), bool(np.any(b2)),
           not bool(np.all(ln1_g == 1)), bool(np.any(ln1_b)),
           not bool(np.all(ln2_g == 1)), bool(np.any(ln2_b)))
    nc = _get_program(key)
    apply_bv, apply_b2, a_g1, a_b1, a_g2, a_b2 = key

    qi = np.arange(128)
    wA = np.where(qi >= 64, NEG, 0.0).astype(_BF16).reshape(1, 128)
    wB = np.where(qi < 64, NEG, 0.0).astype(_BF16).reshape(1, 128)
    wfull = np.full((1, 128), NEG, _BF16)
    uA = (qi < 64).astype(_BF16).reshape(1, 128)
    uB = (qi >= 64).astype(_BF16).reshape(1, 128)

    shared = {
        "wq": Wq.astype(_BF16), "wk": Wk.astype(_BF16), "wv": Wv.astype(_BF16),
        "bqT": np.asarray(bq, np.float32).reshape(4, 128).T.copy(),
        "bkT": np.asarray(bk, np.float32).reshape(4, 128).T.copy(),
        "w1": W1.astype(_BF16),
        "b1T": np.asarray(b1, np.float32).reshape(16, 128).T.copy(),
        "w2": W2.astype(_BF16),
        "ident": np.eye(128, dtype=_BF16),
        "uA": uA, "uB": uB,
    }
    if apply_bv or apply_b2:
        shared["onerow"] = np.ones((1, 128), _BF16)
    if apply_bv:
        shared["bvrow"] = np.asarray(bv, np.float32).reshape(1, D).astype(_BF16)
    if apply_b2:
        shared["b2row"] = np.asarray(b2, np.float32).reshape(1, D).astype(_BF16)
    if a_g1:
        shared["g1"] = np.tile(np.asarray(ln1_g, np.float32).reshape(1, D), (128, 1))
    if a_b1:
        shared["be1"] = np.tile(np.asarray(ln1_b, np.float32).reshape(1, D), (128, 1))
    if a_g2:
        shared["g2"] = np.tile(np.asarray(ln2_g, np.float32).reshape(1, D), (128, 1))
    if a_b2:
        shared["be2"] = np.tile(np.asarray(ln2_b, np.float32).reshape(1, D), (128, 1))

    in_maps = []
    for c in range(8):
        b, h = divmod(c, 2)
        start = h * T - H
        ext = np.zeros((TEXT, D), np.float32)
        lo, hi = max(start, 0), min(start + TEXT, N)
        ext[lo - start: hi - start] = src[b, lo:hi]
        m = dict(shared)
        m["srcT"] = np.ascontiguousarray(ext.T).astype(_BF16)
        m["src"] = np.ascontiguousarray(src[b, h * T:(h + 1) * T])
        m["wA0"] = wfull if h == 0 else wA
        m["wB31"] = wfull if h == 1 else wB
        in_maps.append(m)

    from concourse.bass_utils import run_bass_kernel_spmd
    trace = bool(os.environ.get("KERNEL_TRACE"))
    if trace:
        _install_ntff_hook()
    res = run_bass_kernel_spmd(nc, in_maps, core_ids=list(range(8)), trace=trace)
    if trace:
        last_exec_ns = res.exec_time_ns

    out = np.empty((B, N, D), np.float32)
    for c in range(8):
        b, h = divmod(c, 2)
        out[b, h * T:(h + 1) * T] = res.results[c]["out"]
    return out


# revision 18
# speedup vs baseline: 1.0700x; 1.0388x over previous
"""LocalTransformerEncoderLayer on 8 trn2 NeuronCores.

Sharding: core c = 2*b + h handles batch b, sequence half h (4096 tokens,
plus a 64-token halo on each side for the local-attention window).
Everything is done on-device per core; no collectives needed.

Layout plan (per core):
  srcT  [512, 4224] bf16  d-major haloed chunk (host-transposed)  -> QKV rhs/lhsT
  qT,kT [128,4,4224] bf16 d-major in SBUF (PE: W.T @ srcT)
  v     token-major tiles staged via DRAM scratch (PE: srcT.T @ Wv)
  per q-pair p (128 query tokens, 256 keys = ext tiles p,p+1):
    simT [128keys, 2*128q] psum  = kT.T @ qT (edge masks via rank-1; interior
    window corners zeroed post-exp by gpsimd memsets)
    expT bf16 = ACT exp(scale*simT);  denom = expT.T @ ones (PE);
    av [128q,512] = expT.T @ v;  s = av*recip + src (one DVE op); LN1 stats
  FFN per 512-token block (pipelined 3-4 pairs behind attention):
    xT via PE transpose; h[f,tok] = relu(W1.T @ xT); y[tok,d] = h.T @ W2
    residual2 + LN2 token-major, DMA out fp32.
"""
import os
import numpy as np
import ml_dtypes

_BF16 = ml_dtypes.bfloat16

B, N, D, F, W = 4, 8192, 512, 2048, 64
T = N // 2            # own tokens per core = 4096
H = 64                # halo
TEXT = T + 2 * H      # 4224
NPAIR = T // 128      # 32 q-pairs per core
NBLK = T // 512       # 8 blocks
NEG = -1e10
SCALE = float(D) ** -0.5

_cache = {}


def _build(apply_bv, apply_b2, apply_ln1g, apply_ln1b, apply_ln2g, apply_ln2b):
    import concourse.bacc as bacc
    import concourse.tile as tile
    from concourse import mybir
    import concourse.bass as bass

    f32 = mybir.dt.float32
    bf16 = mybir.dt.bfloat16
    AF = mybir.ActivationFunctionType
    ALU = mybir.AluOpType

    nc = bacc.Bacc("TRN2", target_bir_lowering=False, debug=False)

    # ---- DRAM I/O ----
    srcT_d = nc.dram_tensor("srcT", [D, TEXT], bf16, kind="ExternalInput").ap()
    src_d = nc.dram_tensor("src", [T, D], f32, kind="ExternalInput").ap()
    wq_d = nc.dram_tensor("wq", [D, D], bf16, kind="ExternalInput").ap()
    wk_d = nc.dram_tensor("wk", [D, D], bf16, kind="ExternalInput").ap()
    wv_d = nc.dram_tensor("wv", [D, D], bf16, kind="ExternalInput").ap()
    bqT_d = nc.dram_tensor("bqT", [128, 4], f32, kind="ExternalInput").ap()
    bkT_d = nc.dram_tensor("bkT", [128, 4], f32, kind="ExternalInput").ap()
    w1_d = nc.dram_tensor("w1", [D, F], bf16, kind="ExternalInput").ap()
    b1T_d = nc.dram_tensor("b1T", [128, 16], f32, kind="ExternalInput").ap()
    w2_d = nc.dram_tensor("w2", [F, D], bf16, kind="ExternalInput").ap()
    ident_d = nc.dram_tensor("ident", [128, 128], bf16, kind="ExternalInput").ap()
    uA_d = nc.dram_tensor("uA", [1, 128], bf16, kind="ExternalInput").ap()
    uB_d = nc.dram_tensor("uB", [1, 128], bf16, kind="ExternalInput").ap()
    wA0_d = nc.dram_tensor("wA0", [1, 128], bf16, kind="ExternalInput").ap()
    wB31_d = nc.dram_tensor("wB31", [1, 128], bf16, kind="ExternalInput").ap()
    if apply_bv or apply_b2:
        onerow_d = nc.dram_tensor("onerow", [1, 128], bf16, kind="ExternalInput").ap()
    if apply_bv:
        bvrow_d = nc.dram_tensor("bvrow", [1, D], bf16, kind="ExternalInput").ap()
    if apply_b2:
        b2row_d = nc.dram_tensor("b2row", [1, D], bf16, kind="ExternalInput").ap()
    # replicated LN params (only declared when needed)
    if apply_ln1g:
        g1_d = nc.dram_tensor("g1", [128, D], f32, kind="ExternalInput").ap()
    if apply_ln1b:
        be1_d = nc.dram_tensor("be1", [128, D], f32, kind="ExternalInput").ap()
    if apply_ln2g:
        g2_d = nc.dram_tensor("g2", [128, D], f32, kind="ExternalInput").ap()
    if apply_ln2b:
        be2_d = nc.dram_tensor("be2", [128, D], f32, kind="ExternalInput").ap()
    out_d = nc.dram_tensor("out", [T, D], f32, kind="ExternalOutput").ap()
    v_d = nc.dram_tensor("vscratch", [33 * 128, D], bf16).ap()

    from contextlib import ExitStack
    with tile.TileContext(nc) as tc, ExitStack() as ctx:
        # ---- persistent pools ----
        consts = ctx.enter_context(tc.tile_pool(name="consts", bufs=1))
        kv = ctx.enter_context(tc.tile_pool(name="kv", bufs=1))
        big_ps = ctx.enter_context(tc.tile_pool(name="big_ps", bufs=2, space="PSUM"))
        av_ps = ctx.enter_context(tc.tile_pool(name="av_ps", bufs=2, space="PSUM"))
        sim_ps = ctx.enter_context(tc.tile_pool(name="sim_ps", bufs=2, space="PSUM"))
        xt_ps = ctx.enter_context(tc.tile_pool(name="xt_ps", bufs=2, space="PSUM"))

        # startup-critical constants first: first matmuls need wq kt=0 +
        # srcT block-0 kt=0; split those DMAs per-kt so PE starts early
        wq_sb = consts.tile([128, 4, D], bf16, tag="wq")
        wq_r = wq_d.rearrange("(kt p) m -> p kt m", p=128)
        nc.sync.dma_start(wq_sb[:, 0], wq_r[:, 0])
        bqT_sb = consts.tile([128, 4], f32, tag="bqT")
        nc.sync.dma_start(bqT_sb, bqT_d)

        srcs = ctx.enter_context(tc.tile_pool(name="srcs", bufs=3))
        kv_io = ctx.enter_context(tc.tile_pool(name="kv_io", bufs=3))
        srcT_r = srcT_d.rearrange("(dt p) t -> p dt t", p=128)
        srcT0_sb = srcs.tile([128, 4, 512], bf16, tag="srcT")
        nc.sync.dma_start(srcT0_sb[:, 0], srcT_r[:, 0, 0:512])
        for _kt in range(1, 4):
            nc.sync.dma_start(wq_sb[:, _kt], wq_r[:, _kt])
            nc.sync.dma_start(srcT0_sb[:, _kt], srcT_r[:, _kt, 0:512])

        wk_sb = consts.tile([128, 4, D], bf16, tag="wk")
        nc.sync.dma_start(wk_sb, wk_d.rearrange("(kt p) m -> p kt m", p=128))
        bkT_sb = consts.tile([128, 4], f32, tag="bkT")
        nc.sync.dma_start(bkT_sb, bkT_d)
        wv_sb = consts.tile([128, 4, D], bf16, tag="wv")
        nc.sync.dma_start(wv_sb, wv_d.rearrange("(kt p) m -> p kt m", p=128))

        # remaining constants (small, or needed only later)
        w1_sb = consts.tile([128, 4, F], bf16, tag="w1")
        w2_sb = consts.tile([128, 16, D], bf16, tag="w2")
        b1T_sb = consts.tile([128, 16], f32, tag="b1T")
        nc.sync.dma_start(b1T_sb, b1T_d)
        ident_sb = consts.tile([128, 128], bf16, tag="ident")
        uA_sb = consts.tile([1, 128], bf16, tag="uA")
        uB_sb = consts.tile([1, 128], bf16, tag="uB")
        wA0_sb = consts.tile([1, 128], bf16, tag="wA0")
        wB31_sb = consts.tile([1, 128], bf16, tag="wB31")
        ones_sb = consts.tile([128, 1], bf16, tag="ones")
        nc.vector.memset(ones_sb, 1.0)
        eps_sb = consts.tile([128, 1], f32, tag="eps")
        nc.vector.memset(eps_sb, 1e-5)
        warm_sb = consts.tile([128, 128], bf16, tag="warm")
        nc.vector.memset(warm_sb, 0.0)
        if apply_bv:
            onerow_sb = consts.tile([1, 128], bf16, tag="onerow")
            nc.sync.dma_start(onerow_sb, onerow_d)
            bvrow_sb = consts.tile([1, D], bf16, tag="bvrow")
            nc.sync.dma_start(bvrow_sb, bvrow_d)
        if apply_b2:
            onerow2_sb = consts.tile([1, 128], bf16, tag="onerow2")
            nc.sync.dma_start(onerow2_sb, onerow_d)
            b2row_sb = consts.tile([1, D], bf16, tag="b2row")
            nc.sync.dma_start(b2row_sb, b2row_d)
        if apply_ln1g:
            g1_sb = consts.tile([128, D], f32, tag="g1")
            nc.sync.dma_start(g1_sb, g1_d)
        if apply_ln1b:
            be1_sb = consts.tile([128, D], f32, tag="be1")
            nc.sync.dma_start(be1_sb, be1_d)
        if apply_ln2g:
            g2_sb = consts.tile([128, D], f32, tag="g2")
            nc.sync.dma_start(g2_sb, g2_d)
        if apply_ln2b:
            be2_sb = consts.tile([128, D], f32, tag="be2")
            nc.sync.dma_start(be2_sb, be2_d)

        # persistent activations
        qT_sb = kv.tile([128, 4, TEXT], bf16, tag="qT")
        kT_sb = kv.tile([128, 4, TEXT], bf16, tag="kT")

        # PE warmup during the initial weight/src DMA window: ~40 dummy matmuls
        # keep the HAM activity window busy so real work starts at full clock
        for _w in range(40):
            ps_w = sim_ps.tile([128, 128], f32, tag="sim")
            nc.tensor.matmul(ps_w, lhsT=warm_sb, rhs=warm_sb, start=True, stop=True)

        # ---- phase 1: QKV over ext grid (srcT streamed per block) ----
        blocks = [(i * 512, 512) for i in range(TEXT // 512)] + [(4096, 128)]
        for off, tw in blocks:
            if off == 0:
                srcT_sb = srcT0_sb
            else:
                srcT_sb = srcs.tile([128, 4, 512], bf16, tag="srcT")
                nc.sync.dma_start(srcT_sb[:, :, :tw], srcT_r[:, :, off:off + tw])
            # qT, kT (d-major)
            for w_sb, b_sb, dst in ((wq_sb, bqT_sb, qT_sb), (wk_sb, bkT_sb, kT_sb)):
                for dq in range(4):
                    ps = big_ps.tile([128, 512], f32, tag="big")
                    for kt in range(4):
                        nc.tensor.matmul(
                            ps[:, :tw],
                            lhsT=w_sb[:, kt, dq * 128:(dq + 1) * 128],
                            rhs=srcT_sb[:, kt, :tw],
                            start=(kt == 0), stop=(kt == 3),
                        )
                    nc.scalar.activation(
                        dst[:, dq, off:off + tw], ps[:, :tw],
                        AF.Identity, bias=b_sb[:, dq:dq + 1],
                    )
            # v (token-major), per 128-token tile
            for s in range(tw // 128):
                ti = (off + s * 128) // 128
                ps = big_ps.tile([128, 512], f32, tag="big")
                for kt in range(4):
                    nc.tensor.matmul(
                        ps,
                        lhsT=srcT_sb[:, kt, s * 128:s * 128 + 128],
                        rhs=wv_sb[:, kt, :],
                        start=(kt == 0), stop=(kt == 3 and not apply_bv),
                    )
                if apply_bv:
                    nc.tensor.matmul(ps, lhsT=onerow_sb, rhs=bvrow_sb,
                                     start=False, stop=True)
                v_t = kv_io.tile([128, D], bf16, tag="vout")
                nc.vector.tensor_copy(v_t, ps)
                nc.sync.dma_start(v_d[ti * 128:(ti + 1) * 128, :], v_t)

        # small consts needed at the start of phase 2
        nc.sync.dma_start(uA_sb, uA_d)
        nc.sync.dma_start(uB_sb, uB_d)
        nc.sync.dma_start(wA0_sb, wA0_d)
        nc.sync.dma_start(wB31_sb, wB31_d)
        nc.sync.dma_start(ident_sb, ident_d)

        # ---- phase 2 pools ----
        x_pool = ctx.enter_context(tc.tile_pool(name="x_pool", bufs=9))
        # (x tiles are bf16: feed both the PE transpose and the s2 residual)
        xT_pool = ctx.enter_context(tc.tile_pool(name="xT_pool", bufs=2))
        h_pool = ctx.enter_context(tc.tile_pool(name="h_pool", bufs=1))
        io_pool = ctx.enter_context(tc.tile_pool(name="io_pool", bufs=3))
        out_pool = ctx.enter_context(tc.tile_pool(name="out_pool", bufs=2))
        stat_pool = ctx.enter_context(tc.tile_pool(name="stat_pool", bufs=8))
        vpool = ctx.enter_context(tc.tile_pool(name="vpool", bufs=4))
        exp_pool = ctx.enter_context(tc.tile_pool(name="exp_pool", bufs=4))
        s_pool = ctx.enter_context(tc.tile_pool(name="s_pool", bufs=6))
        s2_pool = ctx.enter_context(tc.tile_pool(name="s2_pool", bufs=4))

        # preload v tiles 0,1 and src tile 0 ahead of the big w1/w2 DMAs
        vts = {}
        src_ts = {}
        for t0 in (0, 1):
            vt = vpool.tile([128, D], bf16, tag="v")
            nc.sync.dma_start(vt, v_d[t0 * 128:(t0 + 1) * 128, :])
            vts[t0] = vt
        st0 = io_pool.tile([128, D], f32, tag="srct")
        nc.sync.dma_start(st0, src_d[0:128, :])
        src_ts[0] = st0

        # FFN weights needed ~10 pair-iterations into phase 2
        nc.sync.dma_start(w1_sb, w1_d.rearrange("(kt p) m -> p kt m", p=128))
        nc.sync.dma_start(w2_sb, w2_d.rearrange("(ft p) m -> p ft m", p=128))


        u32 = mybir.dt.uint32
        MAGIC1 = 0x5f375a86 + 1

        def rsqrt_dve(mv_blk, n, tag):
            """rstd[128,n] = rsqrt(var+eps) on DVE: bit-hack seed + 2 Newton.

            mv_blk packs (mean, var) pairs; vars live at odd columns. Keeps
            Sqrt off the Scalar engine so its ACT table never thrashes
            between Exp/Relu and Sqrt (each reload costs 1.28us).
            """
            if n == 1:
                var_view = mv_blk[:, 1:2]
            else:
                var_view = mv_blk.rearrange("p (n two) -> p n two", two=2)[:, :, 1]
            veps = stat_pool.tile([128, n], f32, tag=tag + "ve", name="veps")
            nc.vector.tensor_scalar_add(veps, var_view, 1e-5)
            # seed bits = MAGIC - (bits(x) >> 1); the subtract must run in the
            # f32 value domain (DVE int add saturates instead of wrapping)
            a = stat_pool.tile([128, n], u32, tag=tag + "a", name="rsq_a")
            nc.vector.tensor_scalar(a, veps.bitcast(u32), 1, None,
                                    ALU.logical_shift_right)
            af = stat_pool.tile([128, n], f32, tag=tag + "af", name="rsq_af")
            nc.vector.tensor_copy(af, a)
            nc.vector.tensor_scalar(af, af, -1.0, float(MAGIC1 - 1),
                                    ALU.mult, ALU.add)
            yb = stat_pool.tile([128, n], u32, tag=tag + "y", name="rsq_y")
            nc.vector.tensor_copy(yb, af)
            y = yb.bitcast(f32)
            t = stat_pool.tile([128, n], f32, tag=tag + "t", name="rsq_t")
            for _ in range(2):
                nc.vector.tensor_tensor(t, y, y, ALU.mult)
                nc.vector.scalar_tensor_tensor(t, t, -0.5, veps, ALU.mult, ALU.mult)
                nc.vector.scalar_tensor_tensor(y, t, 1.5, y, ALU.add, ALU.mult)
            return y

        def ln_norm(dst, s_sb, mean_col, rstd_col, gamma, beta):
            nc.vector.tensor_scalar(dst, s_sb, mean_col, rstd_col,
                                    ALU.subtract, ALU.mult)
            if gamma is not None:
                nc.vector.tensor_mul(dst, dst, gamma)
            if beta is not None:
                nc.vector.tensor_add(dst, dst, beta)

        # ---- phase 2 pipeline state ----
        expT_t = {}
        h_blks = {}
        xbf_blks = {}
        s_tiles = {}
        mv1 = {}
        x_tiles = {}
        xT_blks = {}

        def emit_sim(p):
            qoff = H + p * 128
            if p >= 1:
                vt = vpool.tile([128, D], bf16, tag="v")
                nc.sync.dma_start(vt, v_d[(p + 1) * 128:(p + 2) * 128, :])
                vts[p + 1] = vt
                st = io_pool.tile([128, D], f32, tag="srct")
                nc.sync.dma_start(st, src_d[p * 128:(p + 1) * 128, :])
                src_ts[p] = st
            ps_sim = sim_ps.tile([128, 256], f32, tag="sim")
            for half in (0, 1):
                ktile = p + half
                edge = (uA_sb, wA0_sb) if (half == 0 and p == 0) else \
                       (uB_sb, wB31_sb) if (half == 1 and p == NPAIR - 1) else None
                reg = ps_sim[:, half * 128:(half + 1) * 128]
                for kt in range(4):
                    nc.tensor.matmul(
                        reg,
                        lhsT=kT_sb[:, kt, ktile * 128:(ktile + 1) * 128],
                        rhs=qT_sb[:, kt, qoff:qoff + 128],
                        start=(kt == 0), stop=(kt == 3 and edge is None),
                    )
                if edge is not None:
                    nc.tensor.matmul(reg, lhsT=edge[0], rhs=edge[1],
                                     start=False, stop=True)
            expT = exp_pool.tile([128, 256], bf16, tag="expT")
            nc.scalar.activation(expT, ps_sim, AF.Exp, scale=SCALE)
            # interior window-corner masks: zero after exp (cheaper than rank-1 matmuls)
            if p > 0:
                nc.gpsimd.memset(expT[0:64, 64:128], 0.0)
            if p < NPAIR - 1:
                nc.gpsimd.memset(expT[64:128, 128:192], 0.0)
            expT_t[p] = expT

        def emit_av(p):
            expT = expT_t.pop(p)
            vA = vts.pop(p)
            vB = vts[p + 1]
            ps_den = sim_ps.tile([128, 1], f32, tag="sim")
            nc.tensor.matmul(ps_den, lhsT=expT[:, 0:128], rhs=ones_sb,
                             start=True, stop=False)
            nc.tensor.matmul(ps_den, lhsT=expT[:, 128:256], rhs=ones_sb,
                             start=False, stop=True)
            recip = stat_pool.tile([128, 1], f32, tag="recip")
            nc.vector.reciprocal(recip, ps_den)
            ps_av = av_ps.tile([128, 512], f32, tag="av")
            nc.tensor.matmul(ps_av, lhsT=expT[:, 0:128], rhs=vA,
                             start=True, stop=False)
            nc.tensor.matmul(ps_av, lhsT=expT[:, 128:256], rhs=vB,
                             start=False, stop=True)
            # s = av*recip + src in a single DVE op
            s_sb = s_pool.tile([128, D], f32, tag="s")
            nc.vector.scalar_tensor_tensor(s_sb, ps_av, recip, src_ts.pop(p),
                                           ALU.mult, ALU.add)
            s_tiles[p] = s_sb
            blk, j = divmod(p, 4)
            if j == 0:
                mv1[blk] = stat_pool.tile([128, 8], f32, tag="mv1b", name="mv1b")
            st6 = stat_pool.tile([128, 6], f32, tag="st6")
            nc.vector.bn_stats(st6, s_sb)
            nc.vector.bn_aggr(mv1[blk][:, 2 * j:2 * j + 2], st6)


        def ln_finish(blk):
            mv_blk = mv1.pop(blk)
            rstd1 = rsqrt_dve(mv_blk, 4, "r1")
            for j in range(4):
                p = blk * 4 + j
                x_bf = x_pool.tile([128, D], bf16, tag="xbf")
                ln_norm(x_bf, s_tiles.pop(p), mv_blk[:, 2 * j:2 * j + 1],
                        rstd1[:, j:j + 1],
                        g1_sb if apply_ln1g else None,
                        be1_sb if apply_ln1b else None)
                x_tiles[p] = x_bf

        def emit_transposes(blk):
            xT_blk = xT_pool.tile([128, 4, 512], bf16, tag="xT")
            xT_blks[blk] = xT_blk
            for j in range(4):
                xbf = x_tiles[blk * 4 + j]
                ps_xt = xt_ps.tile([128, 4, 128], bf16, tag="xt")
                for dt in range(4):
                    nc.tensor.transpose(ps_xt[:, dt], xbf[:, dt * 128:(dt + 1) * 128],
                                        ident_sb)
                nc.vector.tensor_copy(xT_blk[:, :, j * 128:(j + 1) * 128], ps_xt)

        def emit_ffn_h(blk):
            xT_blk = xT_blks.pop(blk)
            h_sb = h_pool.tile([128, 16, 512], bf16, tag="h")
            for ft in range(16):
                ps_h = big_ps.tile([128, 512], f32, tag="big")
                for kt in range(4):
                    nc.tensor.matmul(
                        ps_h,
                        lhsT=w1_sb[:, kt, ft * 128:(ft + 1) * 128],
                        rhs=xT_blk[:, kt, :],
                        start=(kt == 0), stop=(kt == 3),
                    )
                nc.scalar.activation(h_sb[:, ft, :], ps_h, AF.Relu,
                                     bias=b1T_sb[:, ft:ft + 1])
            h_blks[blk] = h_sb

        def emit_ffn_y(blk):
            h_sb = h_blks.pop(blk)
            stream = blk == NBLK - 1
            s2_tiles = []
            mv2_blk = stat_pool.tile([128, 8], f32, tag="mv2b", name="mv2b")
            for j in range(4):
                p = blk * 4 + j
                ps_y = big_ps.tile([128, 512], f32, tag="big")
                for ft in range(16):
                    nc.tensor.matmul(
                        ps_y,
                        lhsT=h_sb[:, ft, j * 128:(j + 1) * 128],
                        rhs=w2_sb[:, ft, :],
                        start=(ft == 0), stop=(ft == 15 and not apply_b2),
                    )
                if apply_b2:
                    nc.tensor.matmul(ps_y, lhsT=onerow2_sb, rhs=b2row_sb,
                                     start=False, stop=True)
                s2 = s2_pool.tile([128, D], f32, tag="s2")
                nc.vector.tensor_add(s2, x_tiles.pop(p), ps_y)
                s2_tiles.append(s2)
                st6 = stat_pool.tile([128, 6], f32, tag="st6")
                nc.vector.bn_stats(st6, s2)
                nc.vector.bn_aggr(mv2_blk[:, 2 * j:2 * j + 2], st6)
                if stream:
                    std_j = stat_pool.tile([128, 1], f32, tag="stdj")
                    nc.scalar.activation(std_j, mv2_blk[:, 2 * j + 1:2 * j + 2],
                                         AF.Sqrt, bias=eps_sb)
                    rstd_j = stat_pool.tile([128, 1], f32, tag="rstdj")
                    nc.vector.reciprocal(rstd_j, std_j)
                    o_sb = out_pool.tile([128, D], f32, tag="o")
                    ln_norm(o_sb, s2, mv2_blk[:, 2 * j:2 * j + 1], rstd_j,
                            g2_sb if apply_ln2g else None,
                            be2_sb if apply_ln2b else None)
                    nc.sync.dma_start(out_d[p * 128:(p + 1) * 128, :], o_sb)
            if stream:
                return
            rstd2 = rsqrt_dve(mv2_blk, 4, "r2")
            for j in range(4):
                p = blk * 4 + j
                o_sb = out_pool.tile([128, D], f32, tag="o")
                ln_norm(o_sb, s2_tiles[j], mv2_blk[:, 2 * j:2 * j + 1],
                        rstd2[:, j:j + 1],
                        g2_sb if apply_ln2g else None,
                        be2_sb if apply_ln2b else None)
                nc.sync.dma_start(out_d[p * 128:(p + 1) * 128, :], o_sb)

        # ---- phase 2 pipeline ----
        # av(p-1) | sim(p) | T(blk) at 4b+7 | LN1(blk) at 4b+5 | H at 4b+8 | Y at 4b+9
        for p in range(NPAIR + 6):
            if p < NPAIR:
                emit_sim(p)
            if 1 <= p <= NPAIR:
                emit_av(p - 1)
            if p >= 5 and (p - 5) % 4 == 0 and (p - 5) // 4 < NBLK:
                ln_finish((p - 5) // 4)
            if p >= 7 and (p - 7) % 4 == 0 and (p - 7) // 4 < NBLK:
                emit_transposes((p - 7) // 4)
            if p >= 8 and (p - 8) % 4 == 0:
                emit_ffn_h((p - 8) // 4)
            if p >= 9 and (p - 9) % 4 == 0:
                emit_ffn_y((p - 9) // 4)

    nc.compile()
    return nc


def _get_program(key):
    if key not in _cache:
        _cache[key] = _build(*key)
    return _cache[key]


last_exec_ns = None


def _install_ntff_hook():
    """NTFF profiling hook for axon (normally installed via antenv.axon_hooks)."""
    import sys, types
    if 'antenv.axon_hooks' in sys.modules:
        return
    mod = types.ModuleType('antenv.axon_hooks')
    _h = [None]
    mod.set_axon_ntff_profile_hook = lambda h: _h.__setitem__(0, h)
    mod.get_axon_ntff_profile_hook = lambda: _h[0]
    sys.modules['antenv.axon_hooks'] = mod
    import antenv
    antenv.axon_hooks = mod
    try:
        from trn_agent_boot.trn_boot import _ntff_profile_via_ctypes
        mod.set_axon_ntff_profile_hook(
            _ntff_profile_via_ctypes('/opt/axon/libaxon_pjrt.so'))
    except Exception:
        pass


def kernel(src, mask, Wq, bq, Wk, bk, Wv, bv, ln1_g, ln1_b,
           W1, b1, W2, b2, ln2_g, ln2_b):
    global last_exec_ns
    src = np.asarray(src, np.float32)
    if not bool(np.asarray(mask).all()):
        raise NotImplementedError("only all-true mask supported")

    key = (bool(np.any(bv)), bool(np.any(b2)),
           not bool(np.all(ln1_g == 1)), bool(np.any(ln1_b)),
           not bool(np.all(ln2_g == 1)), bool(np.any(ln2_b)))
    nc = _get_program(key)
    apply_bv, apply_b2, a_g1, a_b1, a_g2, a_b2 = key

    qi = np.arange(128)
    wA = np.where(qi >= 64, NEG, 0.0).astype(_BF16).reshape(1, 128)
    wB = np.where(qi < 64, NEG, 0.0).astype(_BF16).reshape(1, 128)
    wfull = np.full((1, 128), NEG, _BF16)
    uA = (qi < 64).astype(_BF16).reshape(1, 128)
    uB = (qi >= 64).astype(_BF16).reshape(1, 128)

    shared = {
        "wq": Wq.astype(_BF16), "wk": Wk.astype(_BF16), "wv": Wv.astype(_BF16),
        "bqT": np.asarray(bq, np.float32).reshape(4, 128).T.copy(),
        "bkT": np.asarray(bk, np.float32).reshape(4, 128).T.copy(),
        "w1": W1.astype(_BF16),
        "b1T": np.asarray(b1, np.float32).reshape(16, 128).T.copy(),
        "w2": W2.astype(_BF16),
        "ident": np.eye(128, dtype=_BF16),
        "uA": uA, "uB": uB,
    }
    if apply_bv or apply_b2:
        shared["onerow"] = np.ones((1, 128), _BF16)
    if apply_bv:
        shared["bvrow"] = np.asarray(bv, np.float32).reshape(1, D).astype(_BF16)
    if apply_b2:
        shared["b2row"] = np.asarray(b2, np.float32).reshape(1, D).astype(_BF16)
    if a_g1:
        shared["g1"] = np.tile(np.asarray(ln1_g, np.float32).reshape(1, D), (128, 1))
    if a_b1:
        shared["be1"] = np.tile(np.asarray(ln1_b, np.float32).reshape(1, D), (128, 1))
    if a_g2:
        shared["g2"] = np.tile(np.asarray(ln2_g, np.float32).reshape(1, D), (128, 1))
    if a_b2:
        shared["be2"] = np.tile(np.asarray(ln2_b, np.float32).reshape(1, D), (128, 1))

    in_maps = []
    for c in range(8):
        b, h = divmod(c, 2)
        start = h * T - H
        ext = np.zeros((TEXT, D), np.float32)
        lo, hi = max(start, 0), min(start + TEXT, N)
        ext[lo - start: hi - start] = src[b, lo:hi]
        m = dict(shared)
        m["srcT"] = np.ascontiguousarray(ext.T).astype(_BF16)
        m["src"] = np.ascontiguousarray(src[b, h * T:(h + 1) * T])
        m["wA0"] = wfull if h == 0 else wA
        m["wB31"] = wfull if h == 1 else wB
        in_maps.append(m)

    from concourse.bass_utils import run_bass_kernel_spmd
    trace = bool(os.environ.get("KERNEL_TRACE"))
    if trace:
        _install_ntff_hook()
    res = run_bass_kernel_spmd(nc, in_maps, core_ids=list(range(8)), trace=trace)
    if trace:
        last_exec_ns = res.exec_time_ns

    out = np.empty((B, N, D), np.float32)
    for c in range(8):
        b, h = divmod(c, 2)
        out[b, h * T:(h + 1) * T] = res.results[c]["out"]
    return out


# revision 20
# speedup vs baseline: 1.0864x; 1.0154x over previous
"""LocalTransformerEncoderLayer on 8 trn2 NeuronCores.

Sharding: core c = 2*b + h handles batch b, sequence half h (4096 tokens,
plus a 64-token halo on each side for the local-attention window).
Everything is done on-device per core; no collectives needed.

Layout plan (per core):
  srcT  [512, 4224] bf16  d-major haloed chunk (host-transposed)  -> QKV rhs/lhsT
  qT,kT [128,4,4224] bf16 d-major in SBUF (PE: W.T @ srcT)
  v     token-major tiles staged via DRAM scratch (PE: srcT.T @ Wv)
  per q-pair p (128 query tokens, 256 keys = ext tiles p,p+1):
    simT [128keys, 2*128q] psum  = kT.T @ qT (edge masks via rank-1; interior
    window corners zeroed post-exp by gpsimd memsets)
    expT bf16 = ACT exp(scale*simT);  denom = expT.T @ ones (PE);
    av [128q,512] = expT.T @ v;  s = av*recip + src (one DVE op); LN1 stats
  FFN per 512-token block (pipelined 3-4 pairs behind attention):
    xT via PE transpose; h[f,tok] = relu(W1.T @ xT); y[tok,d] = h.T @ W2
    residual2 + LN2 token-major, DMA out fp32.
"""
import os
import numpy as np
import ml_dtypes

_BF16 = ml_dtypes.bfloat16

B, N, D, F, W = 4, 8192, 512, 2048, 64
T = N // 2            # own tokens per core = 4096
H = 64                # halo
TEXT = T + 2 * H      # 4224
NPAIR = T // 128      # 32 q-pairs per core
NBLK = T // 512       # 8 blocks
NEG = -1e10
SCALE = float(D) ** -0.5

_cache = {}


def _build(apply_bv, apply_b2, apply_ln1g, apply_ln1b, apply_ln2g, apply_ln2b):
    import concourse.bacc as bacc
    import concourse.tile as tile
    from concourse import mybir
    import concourse.bass as bass

    f32 = mybir.dt.float32
    bf16 = mybir.dt.bfloat16
    AF = mybir.ActivationFunctionType
    ALU = mybir.AluOpType

    nc = bacc.Bacc("TRN2", target_bir_lowering=False, debug=False)

    # ---- DRAM I/O ----
    srcT_d = nc.dram_tensor("srcT", [D, TEXT], bf16, kind="ExternalInput").ap()
    src_d = nc.dram_tensor("src", [T, D], f32, kind="ExternalInput").ap()
    wq_d = nc.dram_tensor("wq", [D, D], bf16, kind="ExternalInput").ap()
    wk_d = nc.dram_tensor("wk", [D, D], bf16, kind="ExternalInput").ap()
    wv_d = nc.dram_tensor("wv", [D, D], bf16, kind="ExternalInput").ap()
    bqT_d = nc.dram_tensor("bqT", [128, 4], f32, kind="ExternalInput").ap()
    bkT_d = nc.dram_tensor("bkT", [128, 4], f32, kind="ExternalInput").ap()
    w1_d = nc.dram_tensor("w1", [D, F], bf16, kind="ExternalInput").ap()
    b1T_d = nc.dram_tensor("b1T", [128, 16], f32, kind="ExternalInput").ap()
    w2_d = nc.dram_tensor("w2", [F, D], bf16, kind="ExternalInput").ap()
    ident_d = nc.dram_tensor("ident", [128, 128], bf16, kind="ExternalInput").ap()
    uA_d = nc.dram_tensor("uA", [1, 128], bf16, kind="ExternalInput").ap()
    uB_d = nc.dram_tensor("uB", [1, 128], bf16, kind="ExternalInput").ap()
    wA0_d = nc.dram_tensor("wA0", [1, 192], bf16, kind="ExternalInput").ap()
    wB31_d = nc.dram_tensor("wB31", [1, 192], bf16, kind="ExternalInput").ap()
    if apply_bv or apply_b2:
        onerow_d = nc.dram_tensor("onerow", [1, 128], bf16, kind="ExternalInput").ap()
    if apply_bv:
        bvrow_d = nc.dram_tensor("bvrow", [1, D], bf16, kind="ExternalInput").ap()
    if apply_b2:
        b2row_d = nc.dram_tensor("b2row", [1, D], bf16, kind="ExternalInput").ap()
    # replicated LN params (only declared when needed)
    if apply_ln1g:
        g1_d = nc.dram_tensor("g1", [128, D], f32, kind="ExternalInput").ap()
    if apply_ln1b:
        be1_d = nc.dram_tensor("be1", [128, D], f32, kind="ExternalInput").ap()
    if apply_ln2g:
        g2_d = nc.dram_tensor("g2", [128, D], f32, kind="ExternalInput").ap()
    if apply_ln2b:
        be2_d = nc.dram_tensor("be2", [128, D], f32, kind="ExternalInput").ap()
    out_d = nc.dram_tensor("out", [T, D], f32, kind="ExternalOutput").ap()
    v_d = nc.dram_tensor("vscratch", [33 * 128, D], bf16).ap()

    from contextlib import ExitStack
    with tile.TileContext(nc) as tc, ExitStack() as ctx:
        # ---- persistent pools ----
        consts = ctx.enter_context(tc.tile_pool(name="consts", bufs=1))
        kv = ctx.enter_context(tc.tile_pool(name="kv", bufs=1))
        big_ps = ctx.enter_context(tc.tile_pool(name="big_ps", bufs=2, space="PSUM"))
        av_ps = ctx.enter_context(tc.tile_pool(name="av_ps", bufs=2, space="PSUM"))
        sim_ps = ctx.enter_context(tc.tile_pool(name="sim_ps", bufs=2, space="PSUM"))
        xt_ps = ctx.enter_context(tc.tile_pool(name="xt_ps", bufs=2, space="PSUM"))

        # startup-critical constants first: first matmuls need wq kt=0 +
        # srcT block-0 kt=0; split those DMAs per-kt so PE starts early
        wq_sb = consts.tile([128, 4, D], bf16, tag="wq")
        wq_r = wq_d.rearrange("(kt p) m -> p kt m", p=128)
        nc.sync.dma_start(wq_sb[:, 0], wq_r[:, 0])
        bqT_sb = consts.tile([128, 4], f32, tag="bqT")
        nc.sync.dma_start(bqT_sb, bqT_d)

        srcs = ctx.enter_context(tc.tile_pool(name="srcs", bufs=3))
        kv_io = ctx.enter_context(tc.tile_pool(name="kv_io", bufs=3))
        srcT_r = srcT_d.rearrange("(dt p) t -> p dt t", p=128)
        srcT0_sb = srcs.tile([128, 4, 512], bf16, tag="srcT")
        nc.sync.dma_start(srcT0_sb[:, 0], srcT_r[:, 0, 0:512])
        for _kt in range(1, 4):
            nc.sync.dma_start(wq_sb[:, _kt], wq_r[:, _kt])
            nc.sync.dma_start(srcT0_sb[:, _kt], srcT_r[:, _kt, 0:512])

        wk_sb = consts.tile([128, 4, D], bf16, tag="wk")
        nc.sync.dma_start(wk_sb, wk_d.rearrange("(kt p) m -> p kt m", p=128))
        bkT_sb = consts.tile([128, 4], f32, tag="bkT")
        nc.sync.dma_start(bkT_sb, bkT_d)
        wv_sb = consts.tile([128, 4, D], bf16, tag="wv")
        nc.sync.dma_start(wv_sb, wv_d.rearrange("(kt p) m -> p kt m", p=128))

        # remaining constants (small, or needed only later)
        w1_sb = consts.tile([128, 4, F], bf16, tag="w1")
        w2_sb = consts.tile([128, 16, D], bf16, tag="w2")
        b1T_sb = consts.tile([128, 16], f32, tag="b1T")
        nc.sync.dma_start(b1T_sb, b1T_d)
        ident_sb = consts.tile([128, 128], bf16, tag="ident")
        uA_sb = consts.tile([1, 128], bf16, tag="uA")
        uB_sb = consts.tile([1, 128], bf16, tag="uB")
        wA0_sb = consts.tile([1, 192], bf16, tag="wA0")
        wB31_sb = consts.tile([1, 192], bf16, tag="wB31")
        ones_sb = consts.tile([128, 1], bf16, tag="ones")
        nc.vector.memset(ones_sb, 1.0)
        eps_sb = consts.tile([128, 1], f32, tag="eps")
        nc.vector.memset(eps_sb, 1e-5)
        warm_sb = consts.tile([128, 128], bf16, tag="warm")
        nc.vector.memset(warm_sb, 0.0)
        if apply_bv:
            onerow_sb = consts.tile([1, 128], bf16, tag="onerow")
            nc.sync.dma_start(onerow_sb, onerow_d)
            bvrow_sb = consts.tile([1, D], bf16, tag="bvrow")
            nc.sync.dma_start(bvrow_sb, bvrow_d)
        if apply_b2:
            onerow2_sb = consts.tile([1, 128], bf16, tag="onerow2")
            nc.sync.dma_start(onerow2_sb, onerow_d)
            b2row_sb = consts.tile([1, D], bf16, tag="b2row")
            nc.sync.dma_start(b2row_sb, b2row_d)
        if apply_ln1g:
            g1_sb = consts.tile([128, D], f32, tag="g1")
            nc.sync.dma_start(g1_sb, g1_d)
        if apply_ln1b:
            be1_sb = consts.tile([128, D], f32, tag="be1")
            nc.sync.dma_start(be1_sb, be1_d)
        if apply_ln2g:
            g2_sb = consts.tile([128, D], f32, tag="g2")
            nc.sync.dma_start(g2_sb, g2_d)
        if apply_ln2b:
            be2_sb = consts.tile([128, D], f32, tag="be2")
            nc.sync.dma_start(be2_sb, be2_d)

        # persistent activations
        qT_sb = kv.tile([128, 4, TEXT], bf16, tag="qT")
        kT_sb = kv.tile([128, 4, TEXT], bf16, tag="kT")

        # PE warmup during the initial weight/src DMA window: ~40 dummy matmuls
        # keep the HAM activity window busy so real work starts at full clock
        for _w in range(40):
            ps_w = sim_ps.tile([128, 128], f32, tag="sim")
            nc.tensor.matmul(ps_w, lhsT=warm_sb, rhs=warm_sb, start=True, stop=True)

        # ---- phase 1: QKV over ext grid (srcT streamed per block) ----
        blocks = [(i * 512, 512) for i in range(TEXT // 512)] + [(4096, 128)]
        for off, tw in blocks:
            if off == 0:
                srcT_sb = srcT0_sb
            else:
                srcT_sb = srcs.tile([128, 4, 512], bf16, tag="srcT")
                nc.sync.dma_start(srcT_sb[:, :, :tw], srcT_r[:, :, off:off + tw])
            # qT, kT (d-major)
            for w_sb, b_sb, dst in ((wq_sb, bqT_sb, qT_sb), (wk_sb, bkT_sb, kT_sb)):
                for dq in range(4):
                    ps = big_ps.tile([128, 512], f32, tag="big")
                    for kt in range(4):
                        nc.tensor.matmul(
                            ps[:, :tw],
                            lhsT=w_sb[:, kt, dq * 128:(dq + 1) * 128],
                            rhs=srcT_sb[:, kt, :tw],
                            start=(kt == 0), stop=(kt == 3),
                        )
                    nc.scalar.activation(
                        dst[:, dq, off:off + tw], ps[:, :tw],
                        AF.Identity, bias=b_sb[:, dq:dq + 1],
                    )
            # v (token-major), per 128-token tile
            for s in range(tw // 128):
                ti = (off + s * 128) // 128
                ps = big_ps.tile([128, 512], f32, tag="big")
                for kt in range(4):
                    nc.tensor.matmul(
                        ps,
                        lhsT=srcT_sb[:, kt, s * 128:s * 128 + 128],
                        rhs=wv_sb[:, kt, :],
                        start=(kt == 0), stop=(kt == 3 and not apply_bv),
                    )
                if apply_bv:
                    nc.tensor.matmul(ps, lhsT=onerow_sb, rhs=bvrow_sb,
                                     start=False, stop=True)
                v_t = kv_io.tile([128, D], bf16, tag="vout")
                nc.vector.tensor_copy(v_t, ps)
                nc.sync.dma_start(v_d[ti * 128:(ti + 1) * 128, :], v_t)

        # small consts needed at the start of phase 2
        nc.sync.dma_start(uA_sb, uA_d)
        nc.sync.dma_start(uB_sb, uB_d)
        nc.sync.dma_start(wA0_sb, wA0_d)
        nc.sync.dma_start(wB31_sb, wB31_d)
        nc.sync.dma_start(ident_sb, ident_d)

        # ---- phase 2 pools ----
        x_pool = ctx.enter_context(tc.tile_pool(name="x_pool", bufs=9))
        # (x tiles are bf16: feed both the PE transpose and the s2 residual)
        xT_pool = ctx.enter_context(tc.tile_pool(name="xT_pool", bufs=2))
        h_pool = ctx.enter_context(tc.tile_pool(name="h_pool", bufs=1))
        io_pool = ctx.enter_context(tc.tile_pool(name="io_pool", bufs=4))
        out_pool = ctx.enter_context(tc.tile_pool(name="out_pool", bufs=2))
        stat_pool = ctx.enter_context(tc.tile_pool(name="stat_pool", bufs=8))
        vpool = ctx.enter_context(tc.tile_pool(name="vpool", bufs=5))
        exp_pool = ctx.enter_context(tc.tile_pool(name="exp_pool", bufs=4))
        s_pool = ctx.enter_context(tc.tile_pool(name="s_pool", bufs=6))
        s2_pool = ctx.enter_context(tc.tile_pool(name="s2_pool", bufs=4))

        # preload v tiles 0-2 and src tiles 0-1 ahead of the big w1/w2 DMAs
        vts = {}
        src_ts = {}
        for t0 in (0, 1, 2):
            vt = vpool.tile([128, D], bf16, tag="v")
            nc.sync.dma_start(vt, v_d[t0 * 128:(t0 + 1) * 128, :])
            vts[t0] = vt
        for t0 in (0, 1):
            st0 = io_pool.tile([128, D], f32, tag="srct", name="srct0")
            nc.sync.dma_start(st0, src_d[t0 * 128:(t0 + 1) * 128, :])
            src_ts[t0] = st0

        # FFN weights needed ~10 pair-iterations into phase 2
        nc.sync.dma_start(w1_sb, w1_d.rearrange("(kt p) m -> p kt m", p=128))
        nc.sync.dma_start(w2_sb, w2_d.rearrange("(ft p) m -> p ft m", p=128))


        u32 = mybir.dt.uint32
        MAGIC1 = 0x5f375a86 + 1

        def rsqrt_dve(mv_blk, n, tag):
            """rstd[128,n] = rsqrt(var+eps) on DVE: bit-hack seed + 2 Newton.

            mv_blk packs (mean, var) pairs; vars live at odd columns. Keeps
            Sqrt off the Scalar engine so its ACT table never thrashes
            between Exp/Relu and Sqrt (each reload costs 1.28us).
            """
            if n == 1:
                var_view = mv_blk[:, 1:2]
            else:
                var_view = mv_blk.rearrange("p (n two) -> p n two", two=2)[:, :, 1]
            veps = stat_pool.tile([128, n], f32, tag=tag + "ve", name="veps")
            nc.vector.tensor_scalar_add(veps, var_view, 1e-5)
            # seed bits = MAGIC - (bits(x) >> 1); the subtract must run in the
            # f32 value domain (DVE int add saturates instead of wrapping)
            a = stat_pool.tile([128, n], u32, tag=tag + "a", name="rsq_a")
            nc.vector.tensor_scalar(a, veps.bitcast(u32), 1, None,
                                    ALU.logical_shift_right)
            af = stat_pool.tile([128, n], f32, tag=tag + "af", name="rsq_af")
            nc.vector.tensor_copy(af, a)
            nc.vector.tensor_scalar(af, af, -1.0, float(MAGIC1 - 1),
                                    ALU.mult, ALU.add)
            yb = stat_pool.tile([128, n], u32, tag=tag + "y", name="rsq_y")
            nc.vector.tensor_copy(yb, af)
            y = yb.bitcast(f32)
            t = stat_pool.tile([128, n], f32, tag=tag + "t", name="rsq_t")
            for _ in range(2):
                nc.vector.tensor_tensor(t, y, y, ALU.mult)
                nc.vector.scalar_tensor_tensor(t, t, -0.5, veps, ALU.mult, ALU.mult)
                nc.vector.scalar_tensor_tensor(y, t, 1.5, y, ALU.add, ALU.mult)
            return y

        def ln_norm(dst, s_sb, mean_col, rstd_col, gamma, beta):
            nc.vector.tensor_scalar(dst, s_sb, mean_col, rstd_col,
                                    ALU.subtract, ALU.mult)
            if gamma is not None:
                nc.vector.tensor_mul(dst, dst, gamma)
            if beta is not None:
                nc.vector.tensor_add(dst, dst, beta)

        # ---- phase 2 pipeline state ----
        expT_t = {}
        h_blks = {}
        xbf_blks = {}
        s_tiles = {}
        mv1 = {}
        x_tiles = {}
        xT_blks = {}

        def emit_ksim(t):
            """sim for key-tile t vs the 256 queries of pairs {t-1, t}.

            The stationary operand (kT tile) is loaded once per kt and streams
            both adjacent pairs' queries: half the MMs and LDWEIGHTS of the
            per-pair formulation. Edge tiles (t=0, t=NPAIR) stream 192 query
            cols and take a data-driven rank-1 mask (halo padding differs per
            sequence half).
            """
            # prefetch v tile t+2 and residual src tile t+1 (2-iteration lead)
            if t >= 1 and t + 2 <= NPAIR:
                vt = vpool.tile([128, D], bf16, tag="v")
                nc.sync.dma_start(vt, v_d[(t + 2) * 128:(t + 3) * 128, :])
                vts[t + 2] = vt
            if t >= 1 and t + 1 < NPAIR:
                st = io_pool.tile([128, D], f32, tag="srct")
                nc.sync.dma_start(st, src_d[(t + 1) * 128:(t + 2) * 128, :])
                src_ts[t + 1] = st
            if t == 0:
                qlo, qw = 0, 192
            elif t == NPAIR:
                qlo, qw = t * 128 - 64, 192
            else:
                qlo, qw = t * 128 - 64, 256
            edge = (uA_sb, wA0_sb) if t == 0 else \
                   (uB_sb, wB31_sb) if t == NPAIR else None
            ps_sim = sim_ps.tile([128, 256], f32, tag="sim")
            reg = ps_sim[:, :qw]
            for kt in range(4):
                nc.tensor.matmul(
                    reg,
                    lhsT=kT_sb[:, kt, t * 128:(t + 1) * 128],
                    rhs=qT_sb[:, kt, qlo:qlo + qw],
                    start=(kt == 0), stop=(kt == 3 and edge is None),
                )
            if edge is not None:
                nc.tensor.matmul(reg, lhsT=edge[0], rhs=edge[1][:, :qw],
                                 start=False, stop=True)
            expT = exp_pool.tile([128, 256], bf16, tag="expT")
            nc.scalar.activation(expT[:, :qw], ps_sim[:, :qw], AF.Exp, scale=SCALE)
            # interior corner masks, zeroed after exp:
            #   keys[64:128] (KW 2t)   x cols[0:64]    (QW 2t-2)
            #   keys[0:64]   (KW 2t-1) x cols[192:256] (QW 2t+1)
            if 1 <= t <= NPAIR - 1:
                nc.gpsimd.memset(expT[64:128, 0:64], 0.0)
                nc.gpsimd.memset(expT[0:64, 192:256], 0.0)
            expT_t[t] = expT

        def emit_av(p):
            eA = expT_t[p]          # pair p cols in tile p
            eB = expT_t[p + 1]      # pair p cols in tile p+1
            if p == 0:
                sA = eA[:, 64:192]
            else:
                sA = eA[:, 128:256]
            sB = eB[:, 0:128]
            vA = vts.pop(p)
            vB = vts[p + 1]
            ps_den = sim_ps.tile([128, 1], f32, tag="sim")
            nc.tensor.matmul(ps_den, lhsT=sA, rhs=ones_sb,
                             start=True, stop=False)
            nc.tensor.matmul(ps_den, lhsT=sB, rhs=ones_sb,
                             start=False, stop=True)
            recip = stat_pool.tile([128, 1], f32, tag="recip")
            nc.vector.reciprocal(recip, ps_den)
            ps_av = av_ps.tile([128, 512], f32, tag="av")
            nc.tensor.matmul(ps_av, lhsT=sA, rhs=vA,
                             start=True, stop=False)
            nc.tensor.matmul(ps_av, lhsT=sB, rhs=vB,
                             start=False, stop=True)
            expT_t.pop(p)
            # s = av*recip + src in a single DVE op
            s_sb = s_pool.tile([128, D], f32, tag="s")
            nc.vector.scalar_tensor_tensor(s_sb, ps_av, recip, src_ts.pop(p),
                                           ALU.mult, ALU.add)
            s_tiles[p] = s_sb
            blk, j = divmod(p, 4)
            if j == 0:
                mv1[blk] = stat_pool.tile([128, 8], f32, tag="mv1b", name="mv1b")
            st6 = stat_pool.tile([128, 6], f32, tag="st6")
            nc.vector.bn_stats(st6, s_sb)
            nc.vector.bn_aggr(mv1[blk][:, 2 * j:2 * j + 2], st6)


        def ln_finish(blk):
            mv_blk = mv1.pop(blk)
            rstd1 = rsqrt_dve(mv_blk, 4, "r1")
            for j in range(4):
                p = blk * 4 + j
                x_bf = x_pool.tile([128, D], bf16, tag="xbf")
                ln_norm(x_bf, s_tiles.pop(p), mv_blk[:, 2 * j:2 * j + 1],
                        rstd1[:, j:j + 1],
                        g1_sb if apply_ln1g else None,
                        be1_sb if apply_ln1b else None)
                x_tiles[p] = x_bf

        def emit_transposes(blk):
            xT_blk = xT_pool.tile([128, 4, 512], bf16, tag="xT")
            xT_blks[blk] = xT_blk
            for j in range(4):
                xbf = x_tiles[blk * 4 + j]
                ps_xt = xt_ps.tile([128, 4, 128], bf16, tag="xt")
                for dt in range(4):
                    nc.tensor.transpose(ps_xt[:, dt], xbf[:, dt * 128:(dt + 1) * 128],
                                        ident_sb)
                nc.vector.tensor_copy(xT_blk[:, :, j * 128:(j + 1) * 128], ps_xt)

        def emit_ffn_h(blk):
            xT_blk = xT_blks.pop(blk)
            h_sb = h_pool.tile([128, 16, 512], bf16, tag="h")
            for ft in range(16):
                ps_h = big_ps.tile([128, 512], f32, tag="big")
                for kt in range(4):
                    nc.tensor.matmul(
                        ps_h,
                        lhsT=w1_sb[:, kt, ft * 128:(ft + 1) * 128],
                        rhs=xT_blk[:, kt, :],
                        start=(kt == 0), stop=(kt == 3),
                    )
                nc.scalar.activation(h_sb[:, ft, :], ps_h, AF.Relu,
                                     bias=b1T_sb[:, ft:ft + 1])
            h_blks[blk] = h_sb

        def emit_ffn_y(blk):
            h_sb = h_blks.pop(blk)
            stream = blk == NBLK - 1
            s2_tiles = []
            mv2_blk = stat_pool.tile([128, 8], f32, tag="mv2b", name="mv2b")
            for j in range(4):
                p = blk * 4 + j
                ps_y = big_ps.tile([128, 512], f32, tag="big")
                for ft in range(16):
                    nc.tensor.matmul(
                        ps_y,
                        lhsT=h_sb[:, ft, j * 128:(j + 1) * 128],
                        rhs=w2_sb[:, ft, :],
                        start=(ft == 0), stop=(ft == 15 and not apply_b2),
                    )
                if apply_b2:
                    nc.tensor.matmul(ps_y, lhsT=onerow2_sb, rhs=b2row_sb,
                                     start=False, stop=True)
                s2 = s2_pool.tile([128, D], f32, tag="s2")
                nc.vector.tensor_add(s2, x_tiles.pop(p), ps_y)
                s2_tiles.append(s2)
                st6 = stat_pool.tile([128, 6], f32, tag="st6")
                nc.vector.bn_stats(st6, s2)
                nc.vector.bn_aggr(mv2_blk[:, 2 * j:2 * j + 2], st6)
                if stream:
                    std_j = stat_pool.tile([128, 1], f32, tag="stdj")
                    nc.scalar.activation(std_j, mv2_blk[:, 2 * j + 1:2 * j + 2],
                                         AF.Sqrt, bias=eps_sb)
                    rstd_j = stat_pool.tile([128, 1], f32, tag="rstdj")
                    nc.vector.reciprocal(rstd_j, std_j)
                    o_sb = out_pool.tile([128, D], f32, tag="o")
                    ln_norm(o_sb, s2, mv2_blk[:, 2 * j:2 * j + 1], rstd_j,
                            g2_sb if apply_ln2g else None,
                            be2_sb if apply_ln2b else None)
                    nc.sync.dma_start(out_d[p * 128:(p + 1) * 128, :], o_sb)
            if stream:
                return
            rstd2 = rsqrt_dve(mv2_blk, 4, "r2")
            for j in range(4):
                p = blk * 4 + j
                o_sb = out_pool.tile([128, D], f32, tag="o")
                ln_norm(o_sb, s2_tiles[j], mv2_blk[:, 2 * j:2 * j + 1],
                        rstd2[:, j:j + 1],
                        g2_sb if apply_ln2g else None,
                        be2_sb if apply_ln2b else None)
                nc.sync.dma_start(out_d[p * 128:(p + 1) * 128, :], o_sb)

        # ---- phase 2 pipeline ----
        # ksim(p+1) | av(p-1) | T(blk) at 4b+7 | LN1(blk) at 4b+5 | H at 4b+8 | Y at 4b+9
        emit_ksim(0)
        for p in range(NPAIR + 6):
            if p < NPAIR:
                emit_ksim(p + 1)
            if 1 <= p <= NPAIR:
                emit_av(p - 1)
            if p >= 5 and (p - 5) % 4 == 0 and (p - 5) // 4 < NBLK:
                ln_finish((p - 5) // 4)
            if p >= 7 and (p - 7) % 4 == 0 and (p - 7) // 4 < NBLK:
                emit_transposes((p - 7) // 4)
            if p >= 8 and (p - 8) % 4 == 0:
                emit_ffn_h((p - 8) // 4)
            if p >= 9 and (p - 9) % 4 == 0:
                emit_ffn_y((p - 9) // 4)

    nc.compile()
    return nc


def _get_program(key):
    if key not in _cache:
        _cache[key] = _build(*key)
    return _cache[key]


last_exec_ns = None


def _install_ntff_hook():
    """NTFF profiling hook for axon (normally installed via antenv.axon_hooks)."""
    import sys, types
    if 'antenv.axon_hooks' in sys.modules:
        return
    mod = types.ModuleType('antenv.axon_hooks')
    _h = [None]
    mod.set_axon_ntff_profile_hook = lambda h: _h.__setitem__(0, h)
    mod.get_axon_ntff_profile_hook = lambda: _h[0]
    sys.modules['antenv.axon_hooks'] = mod
    import antenv
    antenv.axon_hooks = mod
    try:
        from trn_agent_boot.trn_boot import _ntff_profile_via_ctypes
        mod.set_axon_ntff_profile_hook(
            _ntff_profile_via_ctypes('/opt/axon/libaxon_pjrt.so'))
    except Exception:
        pass


def kernel(src, mask, Wq, bq, Wk, bk, Wv, bv, ln1_g, ln1_b,
           W1, b1, W2, b2, ln2_g, ln2_b):
    global last_exec_ns
    src = np.asarray(src, np.float32)
    if not bool(np.asarray(mask).all()):
        raise NotImplementedError("only all-true mask supported")

    key = (bool(np.any(bv)), bool(np.any(b2)),
           not bool(np.all(ln1_g == 1)), bool(np.any(ln1_b)),
           not bool(np.all(ln2_g == 1)), bool(np.any(ln2_b)))
    nc = _get_program(key)
    apply_bv, apply_b2, a_g1, a_b1, a_g2, a_b2 = key

    qi = np.arange(128)
    cols = np.arange(192)
    # tile 0 (keys[0:64] = KW -1): h==0 -> pad, mask all queries;
    #   h==1 -> real keys, mask only pair-0 qB (cols 128:192, QW 1 two away)
    wA0_h0 = np.full((1, 192), NEG, _BF16)
    wA0_h1 = np.where(cols >= 128, NEG, 0.0).astype(_BF16).reshape(1, 192)
    # tile 32 (keys[64:128] = KW 64): h==1 -> pad, mask all pair-31 queries
    #   (cols 0:128); h==0 -> real keys, mask only cols[0:64] (QW 62 two away)
    wB31_h0 = np.where(cols < 64, NEG, 0.0).astype(_BF16).reshape(1, 192)
    wB31_h1 = np.where(cols < 128, NEG, 0.0).astype(_BF16).reshape(1, 192)
    uA = (qi < 64).astype(_BF16).reshape(1, 128)
    uB = (qi >= 64).astype(_BF16).reshape(1, 128)

    shared = {
        "wq": Wq.astype(_BF16), "wk": Wk.astype(_BF16), "wv": Wv.astype(_BF16),
        "bqT": np.asarray(bq, np.float32).reshape(4, 128).T.copy(),
        "bkT": np.asarray(bk, np.float32).reshape(4, 128).T.copy(),
        "w1": W1.astype(_BF16),
        "b1T": np.asarray(b1, np.float32).reshape(16, 128).T.copy(),
        "w2": W2.astype(_BF16),
        "ident": np.eye(128, dtype=_BF16),
        "uA": uA, "uB": uB,
    }
    if apply_bv or apply_b2:
        shared["onerow"] = np.ones((1, 128), _BF16)
    if apply_bv:
        shared["bvrow"] = np.asarray(bv, np.float32).reshape(1, D).astype(_BF16)
    if apply_b2:
        shared["b2row"] = np.asarray(b2, np.float32).reshape(1, D).astype(_BF16)
    if a_g1:
        shared["g1"] = np.tile(np.asarray(ln1_g, np.float32).reshape(1, D), (128, 1))
    if a_b1:
        shared["be1"] = np.tile(np.asarray(ln1_b, np.float32).reshape(1, D), (128, 1))
    if a_g2:
        shared["g2"] = np.tile(np.asarray(ln2_g, np.float32).reshape(1, D), (128, 1))
    if a_b2:
        shared["be2"] = np.tile(np.asarray(ln2_b, np.float32).reshape(1, D), (128, 1))

    in_maps = []
    for c in range(8):
        b, h = divmod(c, 2)
        start = h * T - H
        ext = np.zeros((TEXT, D), np.float32)
        lo, hi = max(start, 0), min(start + TEXT, N)
        ext[lo - start: hi - start] = src[b, lo:hi]
        m = dict(shared)
        m["srcT"] = np.ascontiguousarray(ext.T).astype(_BF16)
        m["src"] = np.ascontiguousarray(src[b, h * T:(h + 1) * T])
        m["wA0"] = wA0_h0 if h == 0 else wA0_h1
        m["wB31"] = wB31_h1 if h == 1 else wB31_h0
        in_maps.append(m)

    from concourse.bass_utils import run_bass_kernel_spmd
    trace = bool(os.environ.get("KERNEL_TRACE"))
    if trace:
        _install_ntff_hook()
    res = run_bass_kernel_spmd(nc, in_maps, core_ids=list(range(8)), trace=trace)
    if trace:
        last_exec_ns = res.exec_time_ns

    out = np.empty((B, N, D), np.float32)
    for c in range(8):
        b, h = divmod(c, 2)
        out[b, h * T:(h + 1) * T] = res.results[c]["out"]
    return out


# revision 22
# speedup vs baseline: 1.1195x; 1.0305x over previous
"""LocalTransformerEncoderLayer on 8 trn2 NeuronCores.

Sharding: core c = 2*b + h handles batch b, sequence half h (4096 tokens,
plus a 64-token halo on each side for the local-attention window).
Everything is done on-device per core; no collectives needed.

Layout plan (per core):
  srcT  [512, 4224] bf16  d-major haloed chunk (host-transposed)  -> QKV rhs/lhsT
  qT,kT [128,4,4224] bf16 d-major in SBUF (PE: W.T @ srcT)
  v     token-major tiles staged via DRAM scratch (PE: srcT.T @ Wv)
  per q-pair p (128 query tokens, 256 keys = ext tiles p,p+1):
    simT [128keys, 2*128q] psum  = kT.T @ qT (edge masks via rank-1; interior
    window corners zeroed post-exp by gpsimd memsets)
    expT bf16 = ACT exp(scale*simT);  denom = expT.T @ ones (PE);
    av [128q,512] = expT.T @ v;  s = av*recip + src (one DVE op); LN1 stats
  FFN per 512-token block (pipelined 3-4 pairs behind attention):
    xT via PE transpose; h[f,tok] = relu(W1.T @ xT); y[tok,d] = h.T @ W2
    residual2 + LN2 token-major, DMA out fp32.
"""
import os
import numpy as np
import ml_dtypes

_BF16 = ml_dtypes.bfloat16
_F8 = ml_dtypes.float8_e4m3


def _q8(x):
    return np.clip(np.asarray(x, np.float32), -240, 240).astype(_F8)

B, N, D, F, W = 4, 8192, 512, 2048, 64
T = N // 2            # own tokens per core = 4096
H = 64                # halo
TEXT = T + 2 * H      # 4224
NPAIR = T // 128      # 32 q-pairs per core
NBLK = T // 512       # 8 blocks
NEG = -1e10
SCALE = float(D) ** -0.5

_cache = {}


def _build(apply_bv, apply_b2, apply_ln1g, apply_ln1b, apply_ln2g, apply_ln2b):
    import concourse.bacc as bacc
    import concourse.tile as tile
    from concourse import mybir
    import concourse.bass as bass

    f32 = mybir.dt.float32
    bf16 = mybir.dt.bfloat16
    f8 = mybir.dt.float8e4
    DR = mybir.MatmulPerfMode.DoubleRow
    AF = mybir.ActivationFunctionType
    ALU = mybir.AluOpType

    nc = bacc.Bacc("TRN2", target_bir_lowering=False, debug=False)

    # ---- DRAM I/O ----
    srcT_d = nc.dram_tensor("srcT", [D // 2, TEXT], bf16, kind="ExternalInput").ap()
    srcT8_d = nc.dram_tensor("srcT8", [D // 2, TEXT], f8, kind="ExternalInput").ap()
    src_d = nc.dram_tensor("src", [T, D], f32, kind="ExternalInput").ap()
    wq_d = nc.dram_tensor("wq", [D // 2, D], bf16, kind="ExternalInput").ap()
    wq8_d = nc.dram_tensor("wq8", [D // 2, D], f8, kind="ExternalInput").ap()
    wk_d = nc.dram_tensor("wk", [D // 2, D], bf16, kind="ExternalInput").ap()
    wk8_d = nc.dram_tensor("wk8", [D // 2, D], f8, kind="ExternalInput").ap()
    wv_d = nc.dram_tensor("wv", [D // 2, D], bf16, kind="ExternalInput").ap()
    wv8_d = nc.dram_tensor("wv8", [D // 2, D], f8, kind="ExternalInput").ap()
    bqT_d = nc.dram_tensor("bqT", [128, 4], f32, kind="ExternalInput").ap()
    bkT_d = nc.dram_tensor("bkT", [128, 4], f32, kind="ExternalInput").ap()
    w1_d = nc.dram_tensor("w1", [D, F], bf16, kind="ExternalInput").ap()
    b1T_d = nc.dram_tensor("b1T", [128, 16], f32, kind="ExternalInput").ap()
    w2_d = nc.dram_tensor("w2", [F, D], bf16, kind="ExternalInput").ap()
    ident_d = nc.dram_tensor("ident", [128, 128], bf16, kind="ExternalInput").ap()
    uA_d = nc.dram_tensor("uA", [1, 128], bf16, kind="ExternalInput").ap()
    uB_d = nc.dram_tensor("uB", [1, 128], bf16, kind="ExternalInput").ap()
    wA0_d = nc.dram_tensor("wA0", [1, 192], bf16, kind="ExternalInput").ap()
    wB31_d = nc.dram_tensor("wB31", [1, 192], bf16, kind="ExternalInput").ap()
    if apply_bv or apply_b2:
        onerow_d = nc.dram_tensor("onerow", [1, 128], bf16, kind="ExternalInput").ap()
    if apply_bv:
        bvrow_d = nc.dram_tensor("bvrow", [1, D], bf16, kind="ExternalInput").ap()
    if apply_b2:
        b2row_d = nc.dram_tensor("b2row", [1, D], bf16, kind="ExternalInput").ap()
    # replicated LN params (only declared when needed)
    if apply_ln1g:
        g1_d = nc.dram_tensor("g1", [128, D], f32, kind="ExternalInput").ap()
    if apply_ln1b:
        be1_d = nc.dram_tensor("be1", [128, D], f32, kind="ExternalInput").ap()
    if apply_ln2g:
        g2_d = nc.dram_tensor("g2", [128, D], f32, kind="ExternalInput").ap()
    if apply_ln2b:
        be2_d = nc.dram_tensor("be2", [128, D], f32, kind="ExternalInput").ap()
    out_d = nc.dram_tensor("out", [T, D], f32, kind="ExternalOutput").ap()
    v_d = nc.dram_tensor("vscratch", [33 * 128, D], bf16).ap()

    from contextlib import ExitStack
    with tile.TileContext(nc) as tc, ExitStack() as ctx:
        # ---- persistent pools ----
        consts = ctx.enter_context(tc.tile_pool(name="consts", bufs=1))
        kv = ctx.enter_context(tc.tile_pool(name="kv", bufs=1))
        big_ps = ctx.enter_context(tc.tile_pool(name="big_ps", bufs=2, space="PSUM"))
        av_ps = ctx.enter_context(tc.tile_pool(name="av_ps", bufs=2, space="PSUM"))
        sim_ps = ctx.enter_context(tc.tile_pool(name="sim_ps", bufs=2, space="PSUM"))
        xt_ps = ctx.enter_context(tc.tile_pool(name="xt_ps", bufs=2, space="PSUM"))

        # startup-critical constants first: the first q matmul (fp8 DoubleRow
        # over contraction rows 0:256) needs only wq8 + srcT8 block 0 (~256KB)
        wq8_sb = consts.tile([128, 2, D], f8, tag="wq8")
        nc.sync.dma_start(wq8_sb, wq8_d.rearrange("(kt p) m -> p kt m", p=128))
        bqT_sb = consts.tile([128, 4], f32, tag="bqT")
        nc.sync.dma_start(bqT_sb, bqT_d)

        srcs = ctx.enter_context(tc.tile_pool(name="srcs", bufs=3))
        kv_io = ctx.enter_context(tc.tile_pool(name="kv_io", bufs=3))
        srcT_r = srcT_d.rearrange("(dt p) t -> p dt t", p=128)
        srcT8_r = srcT8_d.rearrange("(dt p) t -> p dt t", p=128)
        srcT80_sb = srcs.tile([128, 2, 512], f8, tag="srcT8")
        nc.sync.dma_start(srcT80_sb, srcT8_r[:, :, 0:512])
        wq_sb = consts.tile([128, 2, D], bf16, tag="wq")
        nc.sync.dma_start(wq_sb, wq_d.rearrange("(kt p) m -> p kt m", p=128))
        srcT0_sb = srcs.tile([128, 2, 512], bf16, tag="srcT")
        nc.sync.dma_start(srcT0_sb, srcT_r[:, :, 0:512])

        wk8_sb = consts.tile([128, 2, D], f8, tag="wk8")
        nc.sync.dma_start(wk8_sb, wk8_d.rearrange("(kt p) m -> p kt m", p=128))
        bkT_sb = consts.tile([128, 4], f32, tag="bkT")
        nc.sync.dma_start(bkT_sb, bkT_d)
        wk_sb = consts.tile([128, 2, D], bf16, tag="wk")
        nc.sync.dma_start(wk_sb, wk_d.rearrange("(kt p) m -> p kt m", p=128))
        wv8_sb = consts.tile([128, 2, D], f8, tag="wv8")
        nc.sync.dma_start(wv8_sb, wv8_d.rearrange("(kt p) m -> p kt m", p=128))
        wv_sb = consts.tile([128, 2, D], bf16, tag="wv")
        nc.sync.dma_start(wv_sb, wv_d.rearrange("(kt p) m -> p kt m", p=128))

        # remaining constants (small, or needed only later)
        w1_sb = consts.tile([128, 4, F], bf16, tag="w1")
        w2_sb = consts.tile([128, 16, D], bf16, tag="w2")
        b1T_sb = consts.tile([128, 16], f32, tag="b1T")
        nc.sync.dma_start(b1T_sb, b1T_d)
        ident_sb = consts.tile([128, 128], bf16, tag="ident")
        uA_sb = consts.tile([1, 128], bf16, tag="uA")
        uB_sb = consts.tile([1, 128], bf16, tag="uB")
        wA0_sb = consts.tile([1, 192], bf16, tag="wA0")
        wB31_sb = consts.tile([1, 192], bf16, tag="wB31")
        ones_sb = consts.tile([128, 1], bf16, tag="ones")
        nc.vector.memset(ones_sb, 1.0)
        eps_sb = consts.tile([128, 1], f32, tag="eps")
        nc.vector.memset(eps_sb, 1e-5)
        warm_sb = consts.tile([128, 128], bf16, tag="warm")
        nc.vector.memset(warm_sb, 0.0)
        if apply_bv:
            onerow_sb = consts.tile([1, 128], bf16, tag="onerow")
            nc.sync.dma_start(onerow_sb, onerow_d)
            bvrow_sb = consts.tile([1, D], bf16, tag="bvrow")
            nc.sync.dma_start(bvrow_sb, bvrow_d)
        if apply_b2:
            onerow2_sb = consts.tile([1, 128], bf16, tag="onerow2")
            nc.sync.dma_start(onerow2_sb, onerow_d)
            b2row_sb = consts.tile([1, D], bf16, tag="b2row")
            nc.sync.dma_start(b2row_sb, b2row_d)
        if apply_ln1g:
            g1_sb = consts.tile([128, D], f32, tag="g1")
            nc.sync.dma_start(g1_sb, g1_d)
        if apply_ln1b:
            be1_sb = consts.tile([128, D], f32, tag="be1")
            nc.sync.dma_start(be1_sb, be1_d)
        if apply_ln2g:
            g2_sb = consts.tile([128, D], f32, tag="g2")
            nc.sync.dma_start(g2_sb, g2_d)
        if apply_ln2b:
            be2_sb = consts.tile([128, D], f32, tag="be2")
            nc.sync.dma_start(be2_sb, be2_d)

        # persistent activations
        qT_sb = kv.tile([128, 4, TEXT], bf16, tag="qT")
        kT_sb = kv.tile([128, 4, TEXT], bf16, tag="kT")

        # PE warmup during the initial weight/src DMA window: ~40 dummy matmuls
        # keep the HAM activity window busy so real work starts at full clock
        for _w in range(40):
            ps_w = sim_ps.tile([128, 128], f32, tag="sim")
            nc.tensor.matmul(ps_w, lhsT=warm_sb, rhs=warm_sb, start=True, stop=True)

        # ---- phase 1: QKV over ext grid (srcT streamed per block) ----
        blocks = [(i * 512, 512) for i in range(TEXT // 512)] + [(4096, 128)]
        for off, tw in blocks:
            if off == 0:
                srcT8_sb, srcT_sb = srcT80_sb, srcT0_sb
            else:
                srcT8_sb = srcs.tile([128, 2, 512], f8, tag="srcT8")
                nc.sync.dma_start(srcT8_sb[:, :, :tw], srcT8_r[:, :, off:off + tw])
                srcT_sb = srcs.tile([128, 2, 512], bf16, tag="srcT")
                nc.sync.dma_start(srcT_sb[:, :, :tw], srcT_r[:, :, off:off + tw])
            # qT, kT (d-major): fp8 DoubleRow covers contraction rows 0:256,
            # bf16 covers rows 256:512
            for w8_sb, w_sb, b_sb, dst in ((wq8_sb, wq_sb, bqT_sb, qT_sb),
                                           (wk8_sb, wk_sb, bkT_sb, kT_sb)):
                for dq in range(4):
                    ps = big_ps.tile([128, 512], f32, tag="big")
                    nc.tensor.matmul(
                        ps[:, :tw],
                        lhsT=w8_sb[:, :, dq * 128:(dq + 1) * 128],
                        rhs=srcT8_sb[:, :, :tw],
                        start=True, stop=False, perf_mode=DR,
                    )
                    for kt in range(2):
                        nc.tensor.matmul(
                            ps[:, :tw],
                            lhsT=w_sb[:, kt, dq * 128:(dq + 1) * 128],
                            rhs=srcT_sb[:, kt, :tw],
                            start=False, stop=(kt == 1),
                        )
                    nc.scalar.activation(
                        dst[:, dq, off:off + tw], ps[:, :tw],
                        AF.Identity, bias=b_sb[:, dq:dq + 1],
                    )
            # v (token-major), per 128-token tile
            for s in range(tw // 128):
                ti = (off + s * 128) // 128
                ps = big_ps.tile([128, 512], f32, tag="big")
                nc.tensor.matmul(
                    ps,
                    lhsT=srcT8_sb[:, :, s * 128:s * 128 + 128],
                    rhs=wv8_sb,
                    start=True, stop=False, perf_mode=DR,
                )
                for kt in range(2):
                    nc.tensor.matmul(
                        ps,
                        lhsT=srcT_sb[:, kt, s * 128:s * 128 + 128],
                        rhs=wv_sb[:, kt, :],
                        start=False, stop=(kt == 1 and not apply_bv),
                    )
                if apply_bv:
                    nc.tensor.matmul(ps, lhsT=onerow_sb, rhs=bvrow_sb,
                                     start=False, stop=True)
                v_t = kv_io.tile([128, D], bf16, tag="vout")
                nc.vector.tensor_copy(v_t, ps)
                nc.sync.dma_start(v_d[ti * 128:(ti + 1) * 128, :], v_t)

        # small consts needed at the start of phase 2
        nc.sync.dma_start(uA_sb, uA_d)
        nc.sync.dma_start(uB_sb, uB_d)
        nc.sync.dma_start(wA0_sb, wA0_d)
        nc.sync.dma_start(wB31_sb, wB31_d)
        nc.sync.dma_start(ident_sb, ident_d)

        # ---- phase 2 pools ----
        x_pool = ctx.enter_context(tc.tile_pool(name="x_pool", bufs=9))
        # (x tiles are bf16: feed both the PE transpose and the s2 residual)
        xT_pool = ctx.enter_context(tc.tile_pool(name="xT_pool", bufs=2))
        h_pool = ctx.enter_context(tc.tile_pool(name="h_pool", bufs=1))
        io_pool = ctx.enter_context(tc.tile_pool(name="io_pool", bufs=4))
        out_pool = ctx.enter_context(tc.tile_pool(name="out_pool", bufs=2))
        stat_pool = ctx.enter_context(tc.tile_pool(name="stat_pool", bufs=8))
        vpool = ctx.enter_context(tc.tile_pool(name="vpool", bufs=5))
        exp_pool = ctx.enter_context(tc.tile_pool(name="exp_pool", bufs=4))
        s_pool = ctx.enter_context(tc.tile_pool(name="s_pool", bufs=6))
        s2_pool = ctx.enter_context(tc.tile_pool(name="s2_pool", bufs=4))

        # preload v tiles 0-2 and src tiles 0-1 ahead of the big w1/w2 DMAs
        vts = {}
        src_ts = {}
        for t0 in (0, 1, 2):
            vt = vpool.tile([128, D], bf16, tag="v")
            nc.sync.dma_start(vt, v_d[t0 * 128:(t0 + 1) * 128, :])
            vts[t0] = vt
        for t0 in (0, 1):
            st0 = io_pool.tile([128, D], f32, tag="srct", name="srct0")
            nc.sync.dma_start(st0, src_d[t0 * 128:(t0 + 1) * 128, :])
            src_ts[t0] = st0

        # FFN weights needed ~10 pair-iterations into phase 2
        nc.sync.dma_start(w1_sb, w1_d.rearrange("(kt p) m -> p kt m", p=128))
        nc.sync.dma_start(w2_sb, w2_d.rearrange("(ft p) m -> p ft m", p=128))


        u32 = mybir.dt.uint32
        MAGIC1 = 0x5f375a86 + 1

        def rsqrt_dve(mv_blk, n, tag):
            """rstd[128,n] = rsqrt(var+eps) on DVE: bit-hack seed + 2 Newton.

            mv_blk packs (mean, var) pairs; vars live at odd columns. Keeps
            Sqrt off the Scalar engine so its ACT table never thrashes
            between Exp/Relu and Sqrt (each reload costs 1.28us).
            """
            if n == 1:
                var_view = mv_blk[:, 1:2]
            else:
                var_view = mv_blk.rearrange("p (n two) -> p n two", two=2)[:, :, 1]
            veps = stat_pool.tile([128, n], f32, tag=tag + "ve", name="veps")
            nc.vector.tensor_scalar_add(veps, var_view, 1e-5)
            # seed bits = MAGIC - (bits(x) >> 1); the subtract must run in the
            # f32 value domain (DVE int add saturates instead of wrapping)
            a = stat_pool.tile([128, n], u32, tag=tag + "a", name="rsq_a")
            nc.vector.tensor_scalar(a, veps.bitcast(u32), 1, None,
                                    ALU.logical_shift_right)
            af = stat_pool.tile([128, n], f32, tag=tag + "af", name="rsq_af")
            nc.vector.tensor_copy(af, a)
            nc.vector.tensor_scalar(af, af, -1.0, float(MAGIC1 - 1),
                                    ALU.mult, ALU.add)
            yb = stat_pool.tile([128, n], u32, tag=tag + "y", name="rsq_y")
            nc.vector.tensor_copy(yb, af)
            y = yb.bitcast(f32)
            t = stat_pool.tile([128, n], f32, tag=tag + "t", name="rsq_t")
            for _ in range(2):
                nc.vector.tensor_tensor(t, y, y, ALU.mult)
                nc.vector.scalar_tensor_tensor(t, t, -0.5, veps, ALU.mult, ALU.mult)
                nc.vector.scalar_tensor_tensor(y, t, 1.5, y, ALU.add, ALU.mult)
            return y

        def ln_norm(dst, s_sb, mean_col, rstd_col, gamma, beta):
            nc.vector.tensor_scalar(dst, s_sb, mean_col, rstd_col,
                                    ALU.subtract, ALU.mult)
            if gamma is not None:
                nc.vector.tensor_mul(dst, dst, gamma)
            if beta is not None:
                nc.vector.tensor_add(dst, dst, beta)

        # ---- phase 2 pipeline state ----
        expT_t = {}
        h_blks = {}
        xbf_blks = {}
        s_tiles = {}
        mv1 = {}
        x_tiles = {}
        xT_blks = {}

        def emit_ksim(t):
            """sim for key-tile t vs the 256 queries of pairs {t-1, t}.

            The stationary operand (kT tile) is loaded once per kt and streams
            both adjacent pairs' queries: half the MMs and LDWEIGHTS of the
            per-pair formulation. Edge tiles (t=0, t=NPAIR) stream 192 query
            cols and take a data-driven rank-1 mask (halo padding differs per
            sequence half).
            """
            # prefetch v tile t+2 and residual src tile t+1 (2-iteration lead)
            if t >= 1 and t + 2 <= NPAIR:
                vt = vpool.tile([128, D], bf16, tag="v")
                nc.sync.dma_start(vt, v_d[(t + 2) * 128:(t + 3) * 128, :])
                vts[t + 2] = vt
            if t >= 1 and t + 1 < NPAIR:
                st = io_pool.tile([128, D], f32, tag="srct")
                nc.sync.dma_start(st, src_d[(t + 1) * 128:(t + 2) * 128, :])
                src_ts[t + 1] = st
            if t == 0:
                qlo, qw = 0, 192
            elif t == NPAIR:
                qlo, qw = t * 128 - 64, 192
            else:
                qlo, qw = t * 128 - 64, 256
            edge = (uA_sb, wA0_sb) if t == 0 else \
                   (uB_sb, wB31_sb) if t == NPAIR else None
            ps_sim = sim_ps.tile([128, 256], f32, tag="sim")
            reg = ps_sim[:, :qw]
            for kt in range(4):
                nc.tensor.matmul(
                    reg,
                    lhsT=kT_sb[:, kt, t * 128:(t + 1) * 128],
                    rhs=qT_sb[:, kt, qlo:qlo + qw],
                    start=(kt == 0), stop=(kt == 3 and edge is None),
                )
            if edge is not None:
                nc.tensor.matmul(reg, lhsT=edge[0], rhs=edge[1][:, :qw],
                                 start=False, stop=True)
            expT = exp_pool.tile([128, 256], bf16, tag="expT")
            nc.scalar.activation(expT[:, :qw], ps_sim[:, :qw], AF.Exp, scale=SCALE)
            # interior corner masks, zeroed after exp:
            #   keys[64:128] (KW 2t)   x cols[0:64]    (QW 2t-2)
            #   keys[0:64]   (KW 2t-1) x cols[192:256] (QW 2t+1)
            if 1 <= t <= NPAIR - 1:
                nc.gpsimd.memset(expT[64:128, 0:64], 0.0)
                nc.gpsimd.memset(expT[0:64, 192:256], 0.0)
            expT_t[t] = expT

        def emit_av(p):
            eA = expT_t[p]          # pair p cols in tile p
            eB = expT_t[p + 1]      # pair p cols in tile p+1
            if p == 0:
                sA = eA[:, 64:192]
            else:
                sA = eA[:, 128:256]
            sB = eB[:, 0:128]
            vA = vts.pop(p)
            vB = vts[p + 1]
            ps_den = sim_ps.tile([128, 1], f32, tag="sim")
            nc.tensor.matmul(ps_den, lhsT=sA, rhs=ones_sb,
                             start=True, stop=False)
            nc.tensor.matmul(ps_den, lhsT=sB, rhs=ones_sb,
                             start=False, stop=True)
            recip = stat_pool.tile([128, 1], f32, tag="recip")
            nc.vector.reciprocal(recip, ps_den)
            ps_av = av_ps.tile([128, 512], f32, tag="av")
            nc.tensor.matmul(ps_av, lhsT=sA, rhs=vA,
                             start=True, stop=False)
            nc.tensor.matmul(ps_av, lhsT=sB, rhs=vB,
                             start=False, stop=True)
            expT_t.pop(p)
            # s = av*recip + src in a single DVE op
            s_sb = s_pool.tile([128, D], f32, tag="s")
            nc.vector.scalar_tensor_tensor(s_sb, ps_av, recip, src_ts.pop(p),
                                           ALU.mult, ALU.add)
            s_tiles[p] = s_sb
            blk, j = divmod(p, 4)
            if j == 0:
                mv1[blk] = stat_pool.tile([128, 8], f32, tag="mv1b", name="mv1b")
            st6 = stat_pool.tile([128, 6], f32, tag="st6")
            nc.vector.bn_stats(st6, s_sb)
            nc.vector.bn_aggr(mv1[blk][:, 2 * j:2 * j + 2], st6)


        def ln_finish(blk):
            mv_blk = mv1.pop(blk)
            rstd1 = rsqrt_dve(mv_blk, 4, "r1")
            for j in range(4):
                p = blk * 4 + j
                x_bf = x_pool.tile([128, D], bf16, tag="xbf")
                ln_norm(x_bf, s_tiles.pop(p), mv_blk[:, 2 * j:2 * j + 1],
                        rstd1[:, j:j + 1],
                        g1_sb if apply_ln1g else None,
                        be1_sb if apply_ln1b else None)
                x_tiles[p] = x_bf

        def emit_transposes(blk):
            xT_blk = xT_pool.tile([128, 4, 512], bf16, tag="xT")
            xT_blks[blk] = xT_blk
            for dt in range(4):
                ps_xt = xt_ps.tile([128, 4, 128], bf16, tag="xt")
                for j in range(4):
                    nc.tensor.transpose(
                        ps_xt[:, j],
                        x_tiles[blk * 4 + j][:, dt * 128:(dt + 1) * 128],
                        ident_sb)
                nc.vector.tensor_copy(xT_blk[:, dt, :], ps_xt)

        def emit_ffn_h(blk):
            xT_blk = xT_blks.pop(blk)
            h_sb = h_pool.tile([128, 16, 512], bf16, tag="h")
            for ft in range(16):
                ps_h = big_ps.tile([128, 512], f32, tag="big")
                for kt in range(4):
                    nc.tensor.matmul(
                        ps_h,
                        lhsT=w1_sb[:, kt, ft * 128:(ft + 1) * 128],
                        rhs=xT_blk[:, kt, :],
                        start=(kt == 0), stop=(kt == 3),
                    )
                nc.scalar.activation(h_sb[:, ft, :], ps_h, AF.Relu,
                                     bias=b1T_sb[:, ft:ft + 1])
            h_blks[blk] = h_sb

        def emit_ffn_y(blk):
            h_sb = h_blks.pop(blk)
            stream = blk == NBLK - 1
            s2_tiles = []
            mv2_blk = stat_pool.tile([128, 8], f32, tag="mv2b", name="mv2b")
            for j in range(4):
                p = blk * 4 + j
                ps_y = big_ps.tile([128, 512], f32, tag="big")
                for ft in range(16):
                    nc.tensor.matmul(
                        ps_y,
                        lhsT=h_sb[:, ft, j * 128:(j + 1) * 128],
                        rhs=w2_sb[:, ft, :],
                        start=(ft == 0), stop=(ft == 15 and not apply_b2),
                    )
                if apply_b2:
                    nc.tensor.matmul(ps_y, lhsT=onerow2_sb, rhs=b2row_sb,
                                     start=False, stop=True)
                s2 = s2_pool.tile([128, D], f32, tag="s2")
                nc.vector.tensor_add(s2, x_tiles.pop(p), ps_y)
                s2_tiles.append(s2)
                st6 = stat_pool.tile([128, 6], f32, tag="st6")
                nc.vector.bn_stats(st6, s2)
                nc.vector.bn_aggr(mv2_blk[:, 2 * j:2 * j + 2], st6)
                if stream:
                    std_j = stat_pool.tile([128, 1], f32, tag="stdj")
                    nc.scalar.activation(std_j, mv2_blk[:, 2 * j + 1:2 * j + 2],
                                         AF.Sqrt, bias=eps_sb)
                    rstd_j = stat_pool.tile([128, 1], f32, tag="rstdj")
                    nc.vector.reciprocal(rstd_j, std_j)
                    o_sb = out_pool.tile([128, D], f32, tag="o")
                    ln_norm(o_sb, s2, mv2_blk[:, 2 * j:2 * j + 1], rstd_j,
                            g2_sb if apply_ln2g else None,
                            be2_sb if apply_ln2b else None)
                    nc.sync.dma_start(out_d[p * 128:(p + 1) * 128, :], o_sb)
            if stream:
                return
            rstd2 = rsqrt_dve(mv2_blk, 4, "r2")
            for j in range(4):
                p = blk * 4 + j
                o_sb = out_pool.tile([128, D], f32, tag="o")
                ln_norm(o_sb, s2_tiles[j], mv2_blk[:, 2 * j:2 * j + 1],
                        rstd2[:, j:j + 1],
                        g2_sb if apply_ln2g else None,
                        be2_sb if apply_ln2b else None)
                nc.sync.dma_start(out_d[p * 128:(p + 1) * 128, :], o_sb)

        # ---- phase 2 pipeline ----
        # ksim(p+1) | av(p-1) | T(blk) at 4b+7 | LN1(blk) at 4b+5 | H at 4b+8 | Y at 4b+9
        emit_ksim(0)
        for p in range(NPAIR + 6):
            if p < NPAIR:
                emit_ksim(p + 1)
            if 1 <= p <= NPAIR:
                emit_av(p - 1)
            if p >= 5 and (p - 5) % 4 == 0 and (p - 5) // 4 < NBLK:
                ln_finish((p - 5) // 4)
            if p >= 7 and (p - 7) % 4 == 0 and (p - 7) // 4 < NBLK:
                emit_transposes((p - 7) // 4)
            if p >= 8 and (p - 8) % 4 == 0:
                emit_ffn_h((p - 8) // 4)
            if p >= 9 and (p - 9) % 4 == 0:
                emit_ffn_y((p - 9) // 4)

    nc.compile()
    return nc


def _get_program(key):
    if key not in _cache:
        _cache[key] = _build(*key)
    return _cache[key]


last_exec_ns = None


def _install_ntff_hook():
    """NTFF profiling hook for axon (normally installed via antenv.axon_hooks)."""
    import sys, types
    if 'antenv.axon_hooks' in sys.modules:
        return
    mod = types.ModuleType('antenv.axon_hooks')
    _h = [None]
    mod.set_axon_ntff_profile_hook = lambda h: _h.__setitem__(0, h)
    mod.get_axon_ntff_profile_hook = lambda: _h[0]
    sys.modules['antenv.axon_hooks'] = mod
    import antenv
    antenv.axon_hooks = mod
    try:
        from trn_agent_boot.trn_boot import _ntff_profile_via_ctypes
        mod.set_axon_ntff_profile_hook(
            _ntff_profile_via_ctypes('/opt/axon/libaxon_pjrt.so'))
    except Exception:
        pass


def kernel(src, mask, Wq, bq, Wk, bk, Wv, bv, ln1_g, ln1_b,
           W1, b1, W2, b2, ln2_g, ln2_b):
    global last_exec_ns
    src = np.asarray(src, np.float32)
    if not bool(np.asarray(mask).all()):
        raise NotImplementedError("only all-true mask supported")

    key = (bool(np.any(bv)), bool(np.any(b2)),
           not bool(np.all(ln1_g == 1)), bool(np.any(ln1_b)),
           not bool(np.all(ln2_g == 1)), bool(np.any(ln2_b)))
    nc = _get_program(key)
    apply_bv, apply_b2, a_g1, a_b1, a_g2, a_b2 = key

    qi = np.arange(128)
    cols = np.arange(192)
    # tile 0 (keys[0:64] = KW -1): h==0 -> pad, mask all queries;
    #   h==1 -> real keys, mask only pair-0 qB (cols 128:192, QW 1 two away)
    wA0_h0 = np.full((1, 192), NEG, _BF16)
    wA0_h1 = np.where(cols >= 128, NEG, 0.0).astype(_BF16).reshape(1, 192)
    # tile 32 (keys[64:128] = KW 64): h==1 -> pad, mask all pair-31 queries
    #   (cols 0:128); h==0 -> real keys, mask only cols[0:64] (QW 62 two away)
    wB31_h0 = np.where(cols < 64, NEG, 0.0).astype(_BF16).reshape(1, 192)
    wB31_h1 = np.where(cols < 128, NEG, 0.0).astype(_BF16).reshape(1, 192)
    uA = (qi < 64).astype(_BF16).reshape(1, 128)
    uB = (qi >= 64).astype(_BF16).reshape(1, 128)

    shared = {
        "wq": Wq[256:].astype(_BF16), "wq8": _q8(Wq[:256]),
        "wk": Wk[256:].astype(_BF16), "wk8": _q8(Wk[:256]),
        "wv": Wv[256:].astype(_BF16), "wv8": _q8(Wv[:256]),
        "bqT": np.asarray(bq, np.float32).reshape(4, 128).T.copy(),
        "bkT": np.asarray(bk, np.float32).reshape(4, 128).T.copy(),
        "w1": W1.astype(_BF16),
        "b1T": np.asarray(b1, np.float32).reshape(16, 128).T.copy(),
        "w2": W2.astype(_BF16),
        "ident": np.eye(128, dtype=_BF16),
        "uA": uA, "uB": uB,
    }
    if apply_bv or apply_b2:
        shared["onerow"] = np.ones((1, 128), _BF16)
    if apply_bv:
        shared["bvrow"] = np.asarray(bv, np.float32).reshape(1, D).astype(_BF16)
    if apply_b2:
        shared["b2row"] = np.asarray(b2, np.float32).reshape(1, D).astype(_BF16)
    if a_g1:
        shared["g1"] = np.tile(np.asarray(ln1_g, np.float32).reshape(1, D), (128, 1))
    if a_b1:
        shared["be1"] = np.tile(np.asarray(ln1_b, np.float32).reshape(1, D), (128, 1))
    if a_g2:
        shared["g2"] = np.tile(np.asarray(ln2_g, np.float32).reshape(1, D), (128, 1))
    if a_b2:
        shared["be2"] = np.tile(np.asarray(ln2_b, np.float32).reshape(1, D), (128, 1))

    in_maps = []
    for c in range(8):
        b, h = divmod(c, 2)
        start = h * T - H
        ext = np.zeros((TEXT, D), np.float32)
        lo, hi = max(start, 0), min(start + TEXT, N)
        ext[lo - start: hi - start] = src[b, lo:hi]
        m = dict(shared)
        extT = np.ascontiguousarray(ext.T)
        m["srcT8"] = _q8(extT[:256])
        m["srcT"] = extT[256:].astype(_BF16)
        m["src"] = np.ascontiguousarray(src[b, h * T:(h + 1) * T])
        m["wA0"] = wA0_h0 if h == 0 else wA0_h1
        m["wB31"] = wB31_h1 if h == 1 else wB31_h0
        in_maps.append(m)

    from concourse.bass_utils import run_bass_kernel_spmd
    trace = bool(os.environ.get("KERNEL_TRACE"))
    if trace:
        _install_ntff_hook()
    res = run_bass_kernel_spmd(nc, in_maps, core_ids=list(range(8)), trace=trace)
    if trace:
        last_exec_ns = res.exec_time_ns

    out = np.empty((B, N, D), np.float32)
    for c in range(8):
        b, h = divmod(c, 2)
        out[b, h * T:(h + 1) * T] = res.results[c]["out"]
    return out
